# revision 51
# baseline (speedup 1.0000x reference)
"""Distributed causal self-attention kernel for one TRN2 chip (8 NeuronCores).

Problem: y = CausalSelfAttention(x) with B=2, T=2048, C=1024, 16 heads x 64.

Sharding (per core c = b*4 + hg;  b = batch, hg = head-group of 4 heads):
  - Q/K/V projections: column-sharded per head group (each core computes its
    4 heads' Q,K,V from the full x of its batch).
  - Attention: fully local (4 heads per core), flash-style. Scores are kept
    transposed (s^T[k, q]); the AV matmul emits y in [q, d] orientation
    (65-wide output incl. a ones-column row-sum), so the softmax denominator
    is per-partition and normalization is a cheap per-partition scalar mul.
  - y[q, d] tiles are transposed back to y^T[d, q] on the DMA engines
    (dma_start_transpose), then AllGathered within each batch group of 4
    cores (one gather per (head-pair, t-half), pipelined against compute).
  - o_proj: each core computes its own 256 output columns from the full
    gathered y^T -> output shards are disjoint; the host just concatenates.

All matmuls run in bf16 (fp32 accumulation in PSUM); inputs are converted to
bf16 on the host. QK^T matmuls (contraction dim 64) are packed two-per-PE
via tile_position row tiling.
"""
import sys
sys.path.insert(0, '/opt/trn_rl_repo')
import numpy as np
import ml_dtypes

B, T, C = 2, 2048, 1024
NH, HD = 16, 64
N_CORES = 8
GROUPS = [[0, 1, 2, 3], [4, 5, 6, 7]]
HPC = NH // 4            # heads per core = 4
SH = HPC * HD            # per-core projection width = 256
NCB = C // 128           # contraction blocks = 8
QT = 512                 # query tile
BF16 = ml_dtypes.bfloat16

_CACHE = {}


def _build(t_len):
    import concourse.bass as bass
    import concourse.bacc as bacc
    import concourse.tile as tile
    import concourse.mybir as mybir
    dt = mybir.dt
    f32, bf16 = dt.float32, dt.bfloat16

    nqt = t_len // QT        # query tiles
    ntc = t_len // 128       # t chunks of 128
    VW = HPC * 65            # vhat row width = 260

    nc = bacc.Bacc("TRN2", target_bir_lowering=False, debug=False,
                   num_devices=N_CORES)
    # inputs arrive pre-blocked on the host: [(cblk p) ...] -> [p, cblk*...]
    xT = nc.dram_tensor("xT", [128, NCB * t_len], bf16, kind="ExternalInput")
    wq = nc.dram_tensor("wqT", [128, NCB * SH], bf16, kind="ExternalInput")
    wk = nc.dram_tensor("wkT", [128, NCB * SH], bf16, kind="ExternalInput")
    wv = nc.dram_tensor("wvT", [128, NCB * SH], bf16, kind="ExternalInput")
    wo = nc.dram_tensor("woT", [128, NCB * SH], bf16, kind="ExternalInput")
    masks = nc.dram_tensor("masks", [128, 1024], bf16, kind="ExternalInput")
    out = nc.dram_tensor("out", [SH, t_len], bf16, kind="ExternalOutput")

    n_th = max(1, t_len // 1024)
    th_len = t_len // n_th
    nth = th_len // QT       # q-tiles per t-half

    with tile.TileContext(nc) as tc:
        with tc.tile_pool(name="big", bufs=1) as big, \
             tc.tile_pool(name="epool", bufs=40) as epool, \
             tc.tile_pool(name="small", bufs=8) as small, \
             tc.tile_pool(name="ygp", bufs=8) as ygp, \
             tc.tile_pool(name="stp", bufs=6) as stp, \
             tc.tile_pool(name="ps", bufs=2, space="PSUM") as psp, \
             tc.tile_pool(name="yps", bufs=2, space="PSUM") as yps, \
             tc.tile_pool(name="dram", bufs=1, space="DRAM") as dram:

            # ---- resident SBUF tensors ----
            xt = big.tile([128, NCB * t_len], bf16)       # x^T, c-blocked
            wq_sb = big.tile([128, NCB * SH], bf16)
            wk_sb = big.tile([128, NCB * SH], bf16)
            wv_sb = big.tile([128, NCB * SH], bf16)
            wo_sb = big.tile([128, NCB * SH], bf16)
            mask_sb = big.tile([128, 1024], bf16)
            qt_sb = big.tile([128, 2 * t_len], bf16)      # Q^T, pair-blocked
            kt_sb = big.tile([128, 2 * t_len], bf16)
            vhat_sb = big.tile([128, ntc * VW], bf16)     # [V_h | 1] per head
            yt_sb = big.tile([128, 2 * t_len], bf16)      # y^T, pair-blocked

            # DMA emission in first-consumer order, batched large so the
            # HWDGE's 625ns fixed issue cost stays off the critical path:
            # pair-0 Q/K projection runs k-outer, consuming each x c-block
            # for both Q and K as it lands.
            half = NCB // 2 * SH
            nc.sync.dma_start(wq_sb[:, 0:SH], wq[:, 0:SH])
            nc.sync.dma_start(wk_sb[:, 0:SH], wk[:, 0:SH])
            nc.sync.dma_start(xt[:, 0:t_len // 2], xT[:, 0:t_len // 2])
            nc.sync.dma_start(wq_sb[:, SH:half], wq[:, SH:half])
            nc.sync.dma_start(wk_sb[:, SH:half], wk[:, SH:half])
            nc.sync.dma_start(xt[:, t_len // 2:t_len], xT[:, t_len // 2:t_len])
            for k in range(1, NCB // 2):
                nc.sync.dma_start(xt[:, k * t_len:(k + 1) * t_len],
                                  xT[:, k * t_len:(k + 1) * t_len])
            nc.sync.dma_start(wq_sb[:, half:], wq[:, half:])
            nc.sync.dma_start(wk_sb[:, half:], wk[:, half:])
            for k in range(NCB // 2, NCB):
                nc.sync.dma_start(xt[:, k * t_len:(k + 1) * t_len],
                                  xT[:, k * t_len:(k + 1) * t_len])
            nc.sync.dma_start(wv_sb[:], wv[:])
            nc.sync.dma_start(mask_sb[:], masks[:])
            nc.sync.dma_start(wo_sb[:], wo[:])
            nc.gpsimd.memset(vhat_sb[:], 1.0)

            # ---- DRAM bounce buffers for the AllGathers (pair x q-tile) ----
            agin = [[dram.tile([128, QT], bf16, name=f"agin{p}{n}")
                     for n in range(nqt)] for p in (0, 1)]
            agout = [[dram.tile([512, QT], bf16, name=f"agout{p}{n}")
                      for n in range(nqt)] for p in (0, 1)]

            def qk_proj(pair, w_sb, dst_sb):
                """Q^T/K^T for one head pair: dst rows = head dims (2x64)."""
                for n in range(nqt):
                    ps = psp.tile([128, 1024], f32, name="ps")
                    for k in range(NCB):
                        nc.tensor.matmul(
                            ps[:, 0:QT],
                            lhsT=w_sb[:, k * SH + pair * 128: k * SH + (pair + 1) * 128],
                            rhs=xt[:, k * t_len + n * QT: k * t_len + n * QT + QT],
                            start=(k == 0), stop=(k == NCB - 1))
                    nc.vector.tensor_copy(
                        dst_sb[:, pair * t_len + n * QT: pair * t_len + n * QT + QT],
                        ps[:, 0:QT])

            def qk_proj0_streamed():
                """Pair-0 Q and K projections together, k-outer: every x
                c-block feeds 8 open psum groups (Q n0-3, K n0-3 -- two
                512-col groups per 2-bank tile, one group per bank) so PE
                tracks the incoming x stream instead of waiting for all of
                x before each 512-q tile."""
                tiles = [psp.tile([128, 1024], f32, name="ps") for _ in range(2)] \
                    + [yps.tile([128, 1024], f32, name="yps") for _ in range(2)]
                for k in range(NCB):
                    for i, w_sb in enumerate((wq_sb, wk_sb)):
                        for n in range(nqt):
                            ps = tiles[i * 2 + n // 2]
                            nc.tensor.matmul(
                                ps[:, (n % 2) * QT:(n % 2 + 1) * QT],
                                lhsT=w_sb[:, k * SH: k * SH + 128],
                                rhs=xt[:, k * t_len + n * QT: k * t_len + n * QT + QT],
                                start=(k == 0), stop=(k == NCB - 1),
                                skip_group_check=True)
                for i, dst_sb in enumerate((qt_sb, kt_sb)):
                    for n in range(nqt):
                        # ACT is idle until the first exp: take the psum
                        # drains there so DVE stays free for the v_proj chain
                        nc.scalar.copy(
                            dst_sb[:, n * QT:(n + 1) * QT],
                            tiles[i * 2 + n // 2][:, (n % 2) * QT:(n % 2 + 1) * QT])

            def v_proj(pair, tchs, on_act=False, pool=None):
                """V for one head pair in [t, d] layout, written into vhat
                (cols h*65..h*65+63 per head; col h*65+64 stays 1). Emitted
                in chunks so it can fill PE idle while the exp stream (ACT)
                catches up with the attention matmuls."""
                pool = pool or psp
                for tch in tchs:
                    ps = pool.tile([128, 1024], f32,
                                   name="ps" if pool is psp else "yps")
                    for k in range(NCB):
                        nc.tensor.matmul(
                            ps[:, 0:128],
                            lhsT=xt[:, k * t_len + tch * 128: k * t_len + (tch + 1) * 128],
                            rhs=wv_sb[:, k * SH + pair * 128:k * SH + (pair + 1) * 128],
                            start=(k == 0), stop=(k == NCB - 1))
                    dst = vhat_sb[:, tch * VW + pair * 130: tch * VW + (pair + 1) * 130]
                    cp = nc.scalar.copy if on_act else nc.vector.tensor_copy
                    cp(dst.rearrange("p (h e) -> p h e", e=65)[:, :, 0:64],
                       ps[:, 0:128].rearrange("p (h e) -> p h e", e=64))

            def qk_phase(pair, qi):
                """QK + exp (+ causal mask) for one 512-q tile. Returns the
                e-tiles (scores stay transposed: e[k, q] = exp(s^T)) keyed so
                av_phase can slice per (k-block, 128-q block)."""
                q0 = qi * QT
                nfull = q0 // 128

                def qk_mm(dst, kb, qa, w, h01):
                    nc.tensor.matmul(
                        dst,
                        lhsT=kt_sb[h01 * 64:(h01 + 1) * 64,
                                   pair * t_len + kb * 128: pair * t_len + (kb + 1) * 128],
                        rhs=qt_sb[h01 * 64:(h01 + 1) * 64,
                                  pair * t_len + qa: pair * t_len + qa + w],
                        start=True, stop=True,
                        tile_position=(h01 * 64, 0))

                efull = []
                for kb in range(nfull):
                    qk = psp.tile([128, 1024], f32, name="ps")
                    for h01 in (0, 1):
                        qk_mm(qk[:, h01 * 512:(h01 + 1) * 512], kb, q0, 512, h01)
                    e = epool.tile([128, 1024], bf16, name="e")
                    nc.scalar.activation(e[:], qk[:],
                                         mybir.ActivationFunctionType.Exp,
                                         scale=1.0 / np.sqrt(HD))
                    efull.append(e)
                # mid supertile: blocks nfull, nfull+1 are fully valid for the
                # upper q-half [q0+256, q0+512). Packed (i, h01) x 256.
                mid = psp.tile([128, 1024], f32, name="ps")
                for i in (0, 1):
                    for h01 in (0, 1):
                        qk_mm(mid[:, (h01 * 2 + i) * 256:(h01 * 2 + i + 1) * 256],
                              nfull + i, q0 + 256, 256, h01)
                em = epool.tile([128, 1024], bf16, name="e")
                nc.scalar.activation(em[:], mid[:],
                                     mybir.ActivationFunctionType.Exp,
                                     scale=1.0 / np.sqrt(HD))
                # Two diagonal bands: band u covers q-half [q0+u*256, +256)
                # against k-blocks nfull+2u, nfull+2u+1 with the causal mask.
                ebands = []
                for u in (0, 1):
                    bd = psp.tile([128, 1024], f32, name="ps")
                    for i in (0, 1):
                        for h01 in (0, 1):
                            qk_mm(bd[:, (h01 * 2 + i) * 256:(h01 * 2 + i + 1) * 256],
                                  nfull + 2 * u + i, q0 + u * 256, 256, h01)
                    eb = epool.tile([128, 1024], bf16, name="e")
                    nc.scalar.activation(eb[:], bd[:],
                                         mybir.ActivationFunctionType.Exp,
                                         scale=1.0 / np.sqrt(HD))
                    # only the diagonal quarters (i==j) need the tri mask:
                    # the i<j quarters are fully valid, and the i>j quarters
                    # are skipped by av_phase entirely
                    ebt = eb.rearrange("p (h i j c) -> p h i j c", h=2, i=2, j=2)
                    mt = mask_sb.rearrange("p (h i j c) -> p h i j c", h=2, i=2, j=2)
                    nc.vector.tensor_mul(ebt[:, :, 0, 0], ebt[:, :, 0, 0],
                                         mt[:, :, 0, 0])
                    nc.vector.tensor_mul(ebt[:, :, 1, 1], ebt[:, :, 1, 1],
                                         mt[:, :, 1, 1])
                    ebands.append(eb)
                return efull, em, ebands

            def av_phase(pair, qi, etiles, tail=False):
                """AV + normalize + transpose for one 512-q tile.

                y accumulates in [q, d] orientation; each 128-q block qb gets
                its own [128, 1024] psum tile so the two heads' accumulation
                groups land in different banks (h01 at col 0 / 512) -- a
                start=True matmul clears has_written bits bank-wide, so one
                open group per bank is a hard rule.
                """
                efull, em, ebands = etiles
                q0 = qi * QT
                nfull = q0 // 128

                def av_mm(yq, e_ap, kb, h01, start, stop):
                    h = pair * 2 + h01
                    return nc.tensor.matmul(
                        yq[:, h01 * 512: h01 * 512 + 65],
                        lhsT=e_ap,
                        rhs=vhat_sb[:, kb * VW + h * 65: kb * VW + (h + 1) * 65],
                        start=start, stop=stop,
                        skip_group_check=True)

                def norm(yq, j):
                    """y_norm[q, 2hx64] = y / rowsum; transpose to y^T.
                    On the critical tail (ACT's exp queue already drained),
                    the h01=1 mul and odd-j transposes ride ACT so the
                    normalize chain is two-wide."""
                    rc = small.tile([128, 2], f32, name="rc")
                    nc.vector.reciprocal(
                        rc[:],
                        yq.rearrange("p (h c) -> p h c", h=2)[:, :, 64:65])
                    yn = small.tile([128, 128], bf16, name="yn")
                    for h01 in (0, 1):
                        if tail and h01 == 1:
                            nc.scalar.activation(
                                yn[:, h01 * 64:(h01 + 1) * 64],
                                yq[:, h01 * 512: h01 * 512 + 64],
                                mybir.ActivationFunctionType.Copy,
                                scale=rc[:, h01:h01 + 1])
                        else:
                            nc.vector.tensor_scalar_mul(
                                yn[:, h01 * 64:(h01 + 1) * 64],
                                yq[:, h01 * 512: h01 * 512 + 64],
                                rc[:, h01:h01 + 1])
                    tr = nc.scalar if (tail and j % 2) else nc.sync
                    tr.dma_start_transpose(
                        yt_sb[:, pair * t_len + q0 + j * 128: pair * t_len + q0 + (j + 1) * 128],
                        yn[:])

                # lower q-half: k-blocks 0..nfull-1 (full) + band0
                for j in (0, 1):
                    yq = yps.tile([128, 1024], f32, name="yps")
                    for h01 in (0, 1):
                        for kb in range(nfull):
                            av_mm(yq, efull[kb][:, h01 * 512 + j * 128: h01 * 512 + (j + 1) * 128],
                                  kb, h01, start=(kb == 0), stop=False)
                        for i in (0, 1):
                            if i == 1 and j == 0:   # fully masked quarter
                                continue
                            av_mm(yq, ebands[0][:, (h01 * 2 + i) * 256 + j * 128:
                                                 (h01 * 2 + i) * 256 + (j + 1) * 128],
                                  nfull + i, h01,
                                  start=(nfull == 0 and i == 0),
                                  stop=(i == 1 or j == 0))
                    norm(yq, j)
                # upper q-half: full + mid + band1
                for j in (0, 1):
                    yq = yps.tile([128, 1024], f32, name="yps")
                    for h01 in (0, 1):
                        for kb in range(nfull):
                            av_mm(yq, efull[kb][:, h01 * 512 + (2 + j) * 128:
                                                h01 * 512 + (3 + j) * 128],
                                  kb, h01, start=(kb == 0), stop=False)
                        for i in (0, 1):
                            av_mm(yq, em[:, (h01 * 2 + i) * 256 + j * 128:
                                          (h01 * 2 + i) * 256 + (j + 1) * 128],
                                  nfull + i, h01,
                                  start=(nfull == 0 and i == 0), stop=False)
                        for i in (0, 1):
                            if i == 1 and j == 0:   # fully masked quarter
                                continue
                            av_mm(yq, ebands[1][:, (h01 * 2 + i) * 256 + j * 128:
                                                 (h01 * 2 + i) * 256 + (j + 1) * 128],
                                  nfull + 2 + i, h01, start=False,
                                  stop=(i == 1 or j == 0))
                    norm(yq, 2 + j)

            def gather(p, n):
                """Ship one (pair, 512-q tile) of y^T to the batch group."""
                nc.sync.dma_start(
                    agin[p][n][:],
                    yt_sb[:, p * t_len + n * QT: p * t_len + (n + 1) * QT])
                nc.gpsimd.collective_compute(
                    "AllGather", mybir.AluOpType.bypass,
                    replica_groups=GROUPS,
                    ins=[agin[p][n].opt()], outs=[agout[p][n].opt()])

            ygt = {}  # (global c-block, q-tile) -> sbuf AP [128, 512]

            def load_yg(p, n, eng=None, split=False):
                """One DMA per (pair, q-tile): a 3D AP pulls all 4 ranks'
                [128, 512] blocks at once. split=True issues two halves on
                ACT + SP concurrently to halve the latency on the tail."""
                t = ygp.tile([128, 4 * QT], bf16, name="yg")
                if split:
                    for hf, e in ((0, nc.scalar), (1, nc.sync)):
                        e.dma_start(
                            t[:, hf * 2 * QT:(hf + 1) * 2 * QT].rearrange(
                                "p (r c) -> p r c", r=2),
                            agout[p][n][hf * 256:(hf + 1) * 256, :].rearrange(
                                "(r p) c -> p r c", r=2))
                else:
                    (eng or nc.gpsimd).dma_start(
                        t.rearrange("p (r c) -> p r c", r=4),
                        agout[p][n].rearrange("(r p) c -> p r c", r=4))
                for r in range(4):
                    ygt[(2 * r + p, n)] = t[:, r * QT:(r + 1) * QT]

            deferred_outs = []

            def o_proj_tile(n, cbs, groups, start, stop, defer=False):
                """One 512-t output tile (both 128-o halves), accumulating
                only c-blocks `cbs`; the psum group can stay open across
                calls (start/stop) so the pair-0 half can run before the
                last pair-1 gather lands. defer=True postpones the out DMAs
                so SP stays clear for the latency-critical gather chain."""
                for m in (0, 1):
                    if start:
                        groups[(n, m)] = psp.tile([128, 1024], f32, name="ps")
                    ps = groups[(n, m)]
                    for idx, cb in enumerate(cbs):
                        nc.tensor.matmul(
                            ps[:, 0:QT],
                            lhsT=wo_sb[:, cb * SH + m * 128: cb * SH + (m + 1) * 128],
                            rhs=ygt[(cb, n)],
                            start=(start and idx == 0),
                            stop=(stop and idx == len(cbs) - 1))
                    if stop:
                        st = stp.tile([128, 512], bf16, name="st")
                        nc.vector.tensor_copy(st[:], ps[:, 0:QT])
                        if defer:
                            deferred_outs.append((st, m, n))
                        else:
                            nc.sync.dma_start(
                                out[m * 128:(m + 1) * 128, n * QT: n * QT + QT],
                                st[:])

            # ---- schedule ----
            # Ordered so the serial, ACT-only exp stream never starves and
            # ends on the CHEAPEST tile: q-tiles go descending (the 15-block
            # tile exps while PE still has projection work; the 3-block tile
            # is last so the closing exp -> AV -> gather -> o_proj chain is
            # short). Pair-1 projections + its big qk tile are hoisted into
            # pair-0's attention so ACT crosses the pair boundary without a
            # gap. e-tiles buffer in SBUF (epool) while AV lags several
            # tiles behind QK. Collectives fire per (pair, q-tile) as soon
            # as that tile's y^T ships; o_proj tiles splice into the tail,
            # with the last tile's pair-0 half pre-accumulated under the
            # final gather.
            ogroups = {}
            ALLCB = list(range(NCB))
            EVENCB, ODDCB = [0, 2, 4, 6], [1, 3, 5, 7]

            qk_proj0_streamed()
            v_proj(0, range(0, 8), on_act=True, pool=yps)
            e0 = qk_phase(0, 0)
            e1 = qk_phase(0, 1)
            av_phase(0, 0, e0)
            gather(0, 0)
            e2 = qk_phase(0, 2)
            v_proj(0, range(8, 12))
            av_phase(0, 1, e1)
            gather(0, 1)
            e3 = qk_phase(0, 3)
            v_proj(0, range(12, 16))
            av_phase(0, 2, e2)
            gather(0, 2)
            qk_proj(1, wq_sb, qt_sb)
            av_phase(0, 3, e3)
            gather(0, 3)
            qk_proj(1, wk_sb, kt_sb)
            v_proj(1, range(0, 8))
            f0 = qk_phase(1, 0)
            f1 = qk_phase(1, 1)
            av_phase(1, 0, f0)
            gather(1, 0)
            f2 = qk_phase(1, 2)
            v_proj(1, range(8, 12))
            av_phase(1, 1, f1)
            gather(1, 1)
            f3 = qk_phase(1, 3)
            v_proj(1, range(12, 16))
            av_phase(1, 2, f2)
            gather(1, 2)
            load_yg(1, 0)       # gpsimd; cc(1,0) done by dispatch time
            load_yg(0, 0)
            load_yg(0, 1)
            load_yg(0, 2)
            load_yg(0, 3)
            o_proj_tile(0, ALLCB, ogroups, True, True, defer=True)
            av_phase(1, 3, f3)
            gather(1, 3)
            # late pair-1 tiles load via the ACT DGE: its exp queue drains
            # right as these become needed, and nothing queues behind it
            load_yg(1, 1, eng=nc.scalar)
            o_proj_tile(1, ALLCB, ogroups, True, True, defer=True)
            for st, m, n in deferred_outs:
                nc.sync.dma_start(
                    out[m * 128:(m + 1) * 128, n * QT: n * QT + QT], st[:])
            deferred_outs.clear()
            load_yg(1, 2, eng=nc.scalar)
            o_proj_tile(2, ALLCB, ogroups, True, True)
            # last tile: pair-0 half first; pair-1 blocks land after the
            # final gather, split across two DGEs to halve the latency
            o_proj_tile(3, EVENCB, ogroups, True, False)
            load_yg(1, 3, split=True)
            o_proj_tile(3, ODDCB, ogroups, False, True)

    nc.compile()
    return nc


def _masks_np():
    """Diagonal causal mask: [ki, qi] = qi >= ki, duplicated along the free
    axis for the two packed heads."""
    ki = np.arange(128)[:, None]
    qi = np.arange(128)[None, :]
    tri = (qi >= ki).astype(np.float32)
    ones = np.ones((128, 128), np.float32)
    zeros = np.zeros((128, 128), np.float32)
    lo = np.concatenate([tri, ones], axis=1)    # lower k-block of a band
    hi = np.concatenate([zeros, tri], axis=1)   # upper k-block of a band
    return np.concatenate([lo, hi, lo, hi], axis=1).astype(BF16)  # [128, 1024]


def _block(a, w):
    """[C, w] -> [128, NCB*w] partition-blocked bf16."""
    return np.ascontiguousarray(
        a.reshape(NCB, 128, w).transpose(1, 0, 2).reshape(128, NCB * w)).astype(BF16)


def _prep_inputs(x, Wq, Wk, Wv, Wo, t_len):
    masks = _masks_np()
    in_maps = []
    for c in range(N_CORES):
        b, hg = divmod(c, 4)
        sl = slice(hg * SH, (hg + 1) * SH)
        in_maps.append({
            "xT": _block(x[b].T, t_len),
            "wqT": _block(Wq[sl, :].T, SH),
            "wkT": _block(Wk[sl, :].T, SH),
            "wvT": _block(Wv[sl, :].T, SH),
            "woT": _block(Wo[sl, :].T, SH),
            "masks": masks,
        })
    return in_maps


def _assemble(results, t_len):
    out = np.empty((B, t_len, C), dtype=np.float32)
    for c in range(N_CORES):
        b, hg = divmod(c, 4)
        out[b, :, hg * SH:(hg + 1) * SH] = results[c]["out"].T.astype(np.float32)
    return out


def get_nc(t_len=T):
    if t_len not in _CACHE:
        _CACHE[t_len] = _build(t_len)
    return _CACHE[t_len]


def kernel(x, Wq, Wk, Wv, Wo):
    from concourse import bass_utils
    x = np.asarray(x, dtype=np.float32)
    nc = get_nc(T)
    in_maps = _prep_inputs(x, np.asarray(Wq), np.asarray(Wk), np.asarray(Wv),
                           np.asarray(Wo), T)
    res = bass_utils.run_bass_kernel_spmd(nc, in_maps, core_ids=list(range(N_CORES)))
    return _assemble(res.results, T)


# revision 54
# speedup vs baseline: 1.0133x; 1.0133x over previous
"""Distributed causal self-attention kernel for one TRN2 chip (8 NeuronCores).

Problem: y = CausalSelfAttention(x) with B=2, T=2048, C=1024, 16 heads x 64.

Sharding (per core c = b*4 + hg;  b = batch, hg = head-group of 4 heads):
  - Q/K/V projections: column-sharded per head group (each core computes its
    4 heads' Q,K,V from the full x of its batch).
  - Attention: fully local (4 heads per core), flash-style. Scores are kept
    transposed (s^T[k, q]); the AV matmul emits y in [q, d] orientation
    (65-wide output incl. a ones-column row-sum), so the softmax denominator
    is per-partition and normalization is a cheap per-partition scalar mul.
  - y[q, d] tiles are transposed back to y^T[d, q] on the DMA engines
    (dma_start_transpose), then AllGathered within each batch group of 4
    cores (one gather per (head-pair, t-half), pipelined against compute).
  - o_proj: each core computes its own 256 output columns from the full
    gathered y^T -> output shards are disjoint; the host just concatenates.

All matmuls run in bf16 (fp32 accumulation in PSUM); inputs are converted to
bf16 on the host. QK^T matmuls (contraction dim 64) are packed two-per-PE
via tile_position row tiling.
"""
import sys
sys.path.insert(0, '/opt/trn_rl_repo')
import numpy as np
import ml_dtypes

B, T, C = 2, 2048, 1024
NH, HD = 16, 64
N_CORES = 8
GROUPS = [[0, 1, 2, 3], [4, 5, 6, 7]]
HPC = NH // 4            # heads per core = 4
SH = HPC * HD            # per-core projection width = 256
NCB = C // 128           # contraction blocks = 8
QT = 512                 # query tile
BF16 = ml_dtypes.bfloat16

_CACHE = {}


def _build(t_len):
    import concourse.bass as bass
    import concourse.bacc as bacc
    import concourse.tile as tile
    import concourse.mybir as mybir
    dt = mybir.dt
    f32, bf16 = dt.float32, dt.bfloat16

    nqt = t_len // QT        # query tiles
    ntc = t_len // 128       # t chunks of 128
    VW = HPC * 65            # vhat row width = 260

    nc = bacc.Bacc("TRN2", target_bir_lowering=False, debug=False,
                   num_devices=N_CORES)
    # inputs arrive pre-blocked on the host: [(cblk p) ...] -> [p, cblk*...]
    xT = nc.dram_tensor("xT", [128, NCB * t_len], bf16, kind="ExternalInput")
    wq = nc.dram_tensor("wqT", [128, NCB * SH], bf16, kind="ExternalInput")
    wk = nc.dram_tensor("wkT", [128, NCB * SH], bf16, kind="ExternalInput")
    wv = nc.dram_tensor("wvT", [128, NCB * SH], bf16, kind="ExternalInput")
    wo = nc.dram_tensor("woT", [128, NCB * SH], bf16, kind="ExternalInput")
    masks = nc.dram_tensor("masks", [128, 1024], bf16, kind="ExternalInput")
    out = nc.dram_tensor("out", [SH, t_len], bf16, kind="ExternalOutput")

    n_th = max(1, t_len // 1024)
    th_len = t_len // n_th
    nth = th_len // QT       # q-tiles per t-half

    with tile.TileContext(nc) as tc:
        with tc.tile_pool(name="big", bufs=1) as big, \
             tc.tile_pool(name="epool", bufs=40) as epool, \
             tc.tile_pool(name="small", bufs=8) as small, \
             tc.tile_pool(name="ygp", bufs=8) as ygp, \
             tc.tile_pool(name="stp", bufs=6) as stp, \
             tc.tile_pool(name="ps", bufs=2, space="PSUM") as psp, \
             tc.tile_pool(name="yps", bufs=2, space="PSUM") as yps, \
             tc.tile_pool(name="dram", bufs=1, space="DRAM") as dram:

            # ---- resident SBUF tensors ----
            xt = big.tile([128, NCB * t_len], bf16)       # x^T, c-blocked
            wq_sb = big.tile([128, NCB * SH], bf16)
            wk_sb = big.tile([128, NCB * SH], bf16)
            wv_sb = big.tile([128, NCB * SH], bf16)
            wo_sb = big.tile([128, NCB * SH], bf16)
            mask_sb = big.tile([128, 1024], bf16)
            qt_sb = big.tile([128, 2 * t_len], bf16)      # Q^T, pair-blocked
            kt_sb = big.tile([128, 2 * t_len], bf16)
            vhat_sb = big.tile([128, ntc * VW], bf16)     # [V_h | 1] per head
            yt_sb = big.tile([128, 2 * t_len], bf16)      # y^T, pair-blocked

            # DMA emission in first-consumer order, batched large so the
            # HWDGE's 625ns fixed issue cost stays off the critical path:
            # pair-0 Q/K projection runs k-outer, consuming each x c-block
            # for both Q and K as it lands.
            half = NCB // 2 * SH
            nc.sync.dma_start(wq_sb[:, 0:SH], wq[:, 0:SH])
            nc.sync.dma_start(wk_sb[:, 0:SH], wk[:, 0:SH])
            nc.sync.dma_start(xt[:, 0:t_len // 2], xT[:, 0:t_len // 2])
            nc.sync.dma_start(wq_sb[:, SH:half], wq[:, SH:half])
            nc.sync.dma_start(wk_sb[:, SH:half], wk[:, SH:half])
            nc.sync.dma_start(xt[:, t_len // 2:t_len], xT[:, t_len // 2:t_len])
            for k in range(1, NCB // 2):
                nc.sync.dma_start(xt[:, k * t_len:(k + 1) * t_len],
                                  xT[:, k * t_len:(k + 1) * t_len])
            nc.sync.dma_start(wq_sb[:, half:], wq[:, half:])
            nc.sync.dma_start(wk_sb[:, half:], wk[:, half:])
            for k in range(NCB // 2, NCB):
                nc.sync.dma_start(xt[:, k * t_len:(k + 1) * t_len],
                                  xT[:, k * t_len:(k + 1) * t_len])
            nc.sync.dma_start(wv_sb[:], wv[:])
            nc.sync.dma_start(mask_sb[:], masks[:])
            nc.sync.dma_start(wo_sb[:], wo[:])
            nc.gpsimd.memset(vhat_sb[:], 1.0)

            # ---- DRAM bounce buffers for the AllGathers (pair x q-tile) ----
            agin = [[dram.tile([128, QT], bf16, name=f"agin{p}{n}")
                     for n in range(nqt)] for p in (0, 1)]
            agout = [[dram.tile([512, QT], bf16, name=f"agout{p}{n}")
                      for n in range(nqt)] for p in (0, 1)]

            def qk_proj(pair, w_sb, dst_sb):
                """Q^T/K^T for one head pair: dst rows = head dims (2x64)."""
                for n in range(nqt):
                    ps = psp.tile([128, 1024], f32, name="ps")
                    for k in range(NCB):
                        nc.tensor.matmul(
                            ps[:, 0:QT],
                            lhsT=w_sb[:, k * SH + pair * 128: k * SH + (pair + 1) * 128],
                            rhs=xt[:, k * t_len + n * QT: k * t_len + n * QT + QT],
                            start=(k == 0), stop=(k == NCB - 1))
                    nc.vector.tensor_copy(
                        dst_sb[:, pair * t_len + n * QT: pair * t_len + n * QT + QT],
                        ps[:, 0:QT])

            def qk_proj0_streamed():
                """Pair-0 Q and K projections together, k-outer: every x
                c-block feeds 8 open psum groups (Q n0-3, K n0-3 -- two
                512-col groups per 2-bank tile, one group per bank) so PE
                tracks the incoming x stream instead of waiting for all of
                x before each 512-q tile."""
                tiles = [psp.tile([128, 1024], f32, name="ps") for _ in range(2)] \
                    + [yps.tile([128, 1024], f32, name="yps") for _ in range(2)]
                for k in range(NCB):
                    for i, w_sb in enumerate((wq_sb, wk_sb)):
                        for n in range(nqt):
                            ps = tiles[i * 2 + n // 2]
                            nc.tensor.matmul(
                                ps[:, (n % 2) * QT:(n % 2 + 1) * QT],
                                lhsT=w_sb[:, k * SH: k * SH + 128],
                                rhs=xt[:, k * t_len + n * QT: k * t_len + n * QT + QT],
                                start=(k == 0), stop=(k == NCB - 1),
                                skip_group_check=True)
                for n in range(nqt):
                    for i, dst_sb in enumerate((qt_sb, kt_sb)):
                        # ACT is idle until the first exp: take the psum
                        # drains there so DVE stays free for the v_proj
                        # chain. n-major order so qk_phase(0,0) can start
                        # after the first Q/K pair of copies.
                        nc.scalar.copy(
                            dst_sb[:, n * QT:(n + 1) * QT],
                            tiles[i * 2 + n // 2][:, (n % 2) * QT:(n % 2 + 1) * QT])

            def v_proj(pair, tchs, on_act=False, pool=None):
                """V for one head pair in [t, d] layout, written into vhat
                (cols h*65..h*65+63 per head; col h*65+64 stays 1). Emitted
                in chunks so it can fill PE idle while the exp stream (ACT)
                catches up with the attention matmuls."""
                pool = pool or psp
                for tch in tchs:
                    ps = pool.tile([128, 1024], f32,
                                   name="ps" if pool is psp else "yps")
                    for k in range(NCB):
                        nc.tensor.matmul(
                            ps[:, 0:128],
                            lhsT=xt[:, k * t_len + tch * 128: k * t_len + (tch + 1) * 128],
                            rhs=wv_sb[:, k * SH + pair * 128:k * SH + (pair + 1) * 128],
                            start=(k == 0), stop=(k == NCB - 1))
                    dst = vhat_sb[:, tch * VW + pair * 130: tch * VW + (pair + 1) * 130]
                    cp = nc.scalar.copy if on_act else nc.vector.tensor_copy
                    cp(dst.rearrange("p (h e) -> p h e", e=65)[:, :, 0:64],
                       ps[:, 0:128].rearrange("p (h e) -> p h e", e=64))

            def qk_phase(pair, qi):
                """QK + exp (+ causal mask) for one 512-q tile. Returns the
                e-tiles (scores stay transposed: e[k, q] = exp(s^T)) keyed so
                av_phase can slice per (k-block, 128-q block)."""
                q0 = qi * QT
                nfull = q0 // 128

                def qk_mm(dst, kb, qa, w, h01):
                    nc.tensor.matmul(
                        dst,
                        lhsT=kt_sb[h01 * 64:(h01 + 1) * 64,
                                   pair * t_len + kb * 128: pair * t_len + (kb + 1) * 128],
                        rhs=qt_sb[h01 * 64:(h01 + 1) * 64,
                                  pair * t_len + qa: pair * t_len + qa + w],
                        start=True, stop=True,
                        tile_position=(h01 * 64, 0))

                efull = []
                for kb in range(nfull):
                    qk = psp.tile([128, 1024], f32, name="ps")
                    for h01 in (0, 1):
                        qk_mm(qk[:, h01 * 512:(h01 + 1) * 512], kb, q0, 512, h01)
                    e = epool.tile([128, 1024], bf16, name="e")
                    nc.scalar.activation(e[:], qk[:],
                                         mybir.ActivationFunctionType.Exp,
                                         scale=1.0 / np.sqrt(HD))
                    efull.append(e)
                # mid supertile: blocks nfull, nfull+1 are fully valid for the
                # upper q-half [q0+256, q0+512). Packed (i, h01) x 256.
                mid = psp.tile([128, 1024], f32, name="ps")
                for i in (0, 1):
                    for h01 in (0, 1):
                        qk_mm(mid[:, (h01 * 2 + i) * 256:(h01 * 2 + i + 1) * 256],
                              nfull + i, q0 + 256, 256, h01)
                em = epool.tile([128, 1024], bf16, name="e")
                nc.scalar.activation(em[:], mid[:],
                                     mybir.ActivationFunctionType.Exp,
                                     scale=1.0 / np.sqrt(HD))
                # Two diagonal bands: band u covers q-half [q0+u*256, +256)
                # against k-blocks nfull+2u, nfull+2u+1 with the causal mask.
                ebands = []
                for u in (0, 1):
                    bd = psp.tile([128, 1024], f32, name="ps")
                    for i in (0, 1):
                        for h01 in (0, 1):
                            qk_mm(bd[:, (h01 * 2 + i) * 256:(h01 * 2 + i + 1) * 256],
                                  nfull + 2 * u + i, q0 + u * 256, 256, h01)
                    eb = epool.tile([128, 1024], bf16, name="e")
                    nc.scalar.activation(eb[:], bd[:],
                                         mybir.ActivationFunctionType.Exp,
                                         scale=1.0 / np.sqrt(HD))
                    # only the diagonal quarters (i==j) need the tri mask:
                    # the i<j quarters are fully valid, and the i>j quarters
                    # are skipped by av_phase entirely
                    ebt = eb.rearrange("p (h i j c) -> p h i j c", h=2, i=2, j=2)
                    mt = mask_sb.rearrange("p (h i j c) -> p h i j c", h=2, i=2, j=2)
                    nc.vector.tensor_mul(ebt[:, :, 0, 0], ebt[:, :, 0, 0],
                                         mt[:, :, 0, 0])
                    nc.vector.tensor_mul(ebt[:, :, 1, 1], ebt[:, :, 1, 1],
                                         mt[:, :, 1, 1])
                    ebands.append(eb)
                return efull, em, ebands

            def av_phase(pair, qi, etiles, tail=False):
                """AV + normalize + transpose for one 512-q tile.

                y accumulates in [q, d] orientation; each 128-q block qb gets
                its own [128, 1024] psum tile so the two heads' accumulation
                groups land in different banks (h01 at col 0 / 512) -- a
                start=True matmul clears has_written bits bank-wide, so one
                open group per bank is a hard rule.
                """
                efull, em, ebands = etiles
                q0 = qi * QT
                nfull = q0 // 128

                def av_mm(yq, e_ap, kb, h01, start, stop):
                    h = pair * 2 + h01
                    return nc.tensor.matmul(
                        yq[:, h01 * 512: h01 * 512 + 65],
                        lhsT=e_ap,
                        rhs=vhat_sb[:, kb * VW + h * 65: kb * VW + (h + 1) * 65],
                        start=start, stop=stop,
                        skip_group_check=True)

                def norm(yq, j):
                    """y_norm[q, 2hx64] = y / rowsum; transpose to y^T.
                    On the critical tail (ACT's exp queue already drained),
                    the h01=1 mul and odd-j transposes ride ACT so the
                    normalize chain is two-wide."""
                    rc = small.tile([128, 2], f32, name="rc")
                    nc.vector.reciprocal(
                        rc[:],
                        yq.rearrange("p (h c) -> p h c", h=2)[:, :, 64:65])
                    yn = small.tile([128, 128], bf16, name="yn")
                    for h01 in (0, 1):
                        if tail and h01 == 1:
                            nc.scalar.activation(
                                yn[:, h01 * 64:(h01 + 1) * 64],
                                yq[:, h01 * 512: h01 * 512 + 64],
                                mybir.ActivationFunctionType.Copy,
                                scale=rc[:, h01:h01 + 1])
                        else:
                            nc.vector.tensor_scalar_mul(
                                yn[:, h01 * 64:(h01 + 1) * 64],
                                yq[:, h01 * 512: h01 * 512 + 64],
                                rc[:, h01:h01 + 1])
                    tr = nc.scalar if (tail and j % 2) else nc.sync
                    tr.dma_start_transpose(
                        yt_sb[:, pair * t_len + q0 + j * 128: pair * t_len + q0 + (j + 1) * 128],
                        yn[:])

                # lower q-half: k-blocks 0..nfull-1 (full) + band0
                for j in (0, 1):
                    yq = yps.tile([128, 1024], f32, name="yps")
                    for h01 in (0, 1):
                        for kb in range(nfull):
                            av_mm(yq, efull[kb][:, h01 * 512 + j * 128: h01 * 512 + (j + 1) * 128],
                                  kb, h01, start=(kb == 0), stop=False)
                        for i in (0, 1):
                            if i == 1 and j == 0:   # fully masked quarter
                                continue
                            av_mm(yq, ebands[0][:, (h01 * 2 + i) * 256 + j * 128:
                                                 (h01 * 2 + i) * 256 + (j + 1) * 128],
                                  nfull + i, h01,
                                  start=(nfull == 0 and i == 0),
                                  stop=(i == 1 or j == 0))
                    norm(yq, j)
                # upper q-half: full + mid + band1
                for j in (0, 1):
                    yq = yps.tile([128, 1024], f32, name="yps")
                    for h01 in (0, 1):
                        for kb in range(nfull):
                            av_mm(yq, efull[kb][:, h01 * 512 + (2 + j) * 128:
                                                h01 * 512 + (3 + j) * 128],
                                  kb, h01, start=(kb == 0), stop=False)
                        for i in (0, 1):
                            av_mm(yq, em[:, (h01 * 2 + i) * 256 + j * 128:
                                          (h01 * 2 + i) * 256 + (j + 1) * 128],
                                  nfull + i, h01,
                                  start=(nfull == 0 and i == 0), stop=False)
                        for i in (0, 1):
                            if i == 1 and j == 0:   # fully masked quarter
                                continue
                            av_mm(yq, ebands[1][:, (h01 * 2 + i) * 256 + j * 128:
                                                 (h01 * 2 + i) * 256 + (j + 1) * 128],
                                  nfull + 2 + i, h01, start=False,
                                  stop=(i == 1 or j == 0))
                    norm(yq, 2 + j)

            def gather(p, n):
                """Ship one (pair, 512-q tile) of y^T to the batch group."""
                nc.sync.dma_start(
                    agin[p][n][:],
                    yt_sb[:, p * t_len + n * QT: p * t_len + (n + 1) * QT])
                nc.gpsimd.collective_compute(
                    "AllGather", mybir.AluOpType.bypass,
                    replica_groups=GROUPS,
                    ins=[agin[p][n].opt()], outs=[agout[p][n].opt()])

            ygt = {}  # (global c-block, q-tile) -> sbuf AP [128, 512]

            def load_yg(p, n, eng=None, split=False):
                """One DMA per (pair, q-tile): a 3D AP pulls all 4 ranks'
                [128, 512] blocks at once. split=True issues two halves on
                ACT + SP concurrently to halve the latency on the tail."""
                t = ygp.tile([128, 4 * QT], bf16, name="yg")
                if split:
                    for hf, e in ((0, nc.scalar), (1, nc.sync)):
                        e.dma_start(
                            t[:, hf * 2 * QT:(hf + 1) * 2 * QT].rearrange(
                                "p (r c) -> p r c", r=2),
                            agout[p][n][hf * 256:(hf + 1) * 256, :].rearrange(
                                "(r p) c -> p r c", r=2))
                else:
                    (eng or nc.gpsimd).dma_start(
                        t.rearrange("p (r c) -> p r c", r=4),
                        agout[p][n].rearrange("(r p) c -> p r c", r=4))
                for r in range(4):
                    ygt[(2 * r + p, n)] = t[:, r * QT:(r + 1) * QT]

            deferred_outs = []

            def o_proj_tile(n, cbs, groups, start, stop, defer=False):
                """One 512-t output tile (both 128-o halves), accumulating
                only c-blocks `cbs`; the psum group can stay open across
                calls (start/stop) so the pair-0 half can run before the
                last pair-1 gather lands. defer=True postpones the out DMAs
                so SP stays clear for the latency-critical gather chain."""
                for m in (0, 1):
                    if start:
                        groups[(n, m)] = psp.tile([128, 1024], f32, name="ps")
                    ps = groups[(n, m)]
                    for idx, cb in enumerate(cbs):
                        nc.tensor.matmul(
                            ps[:, 0:QT],
                            lhsT=wo_sb[:, cb * SH + m * 128: cb * SH + (m + 1) * 128],
                            rhs=ygt[(cb, n)],
                            start=(start and idx == 0),
                            stop=(stop and idx == len(cbs) - 1))
                    if stop:
                        st = stp.tile([128, 512], bf16, name="st")
                        nc.vector.tensor_copy(st[:], ps[:, 0:QT])
                        if defer:
                            deferred_outs.append((st, m, n))
                        else:
                            nc.sync.dma_start(
                                out[m * 128:(m + 1) * 128, n * QT: n * QT + QT],
                                st[:])

            # ---- schedule ----
            # Ordered so the serial, ACT-only exp stream never starves and
            # ends on the CHEAPEST tile: q-tiles go descending (the 15-block
            # tile exps while PE still has projection work; the 3-block tile
            # is last so the closing exp -> AV -> gather -> o_proj chain is
            # short). Pair-1 projections + its big qk tile are hoisted into
            # pair-0's attention so ACT crosses the pair boundary without a
            # gap. e-tiles buffer in SBUF (epool) while AV lags several
            # tiles behind QK. Collectives fire per (pair, q-tile) as soon
            # as that tile's y^T ships; o_proj tiles splice into the tail,
            # with the last tile's pair-0 half pre-accumulated under the
            # final gather.
            ogroups = {}
            ALLCB = list(range(NCB))
            EVENCB, ODDCB = [0, 2, 4, 6], [1, 3, 5, 7]

            qk_proj0_streamed()
            v_proj(0, range(0, 8), on_act=True, pool=yps)
            e0 = qk_phase(0, 0)
            e1 = qk_phase(0, 1)
            av_phase(0, 0, e0)
            gather(0, 0)
            e2 = qk_phase(0, 2)
            v_proj(0, range(8, 12))
            av_phase(0, 1, e1)
            gather(0, 1)
            e3 = qk_phase(0, 3)
            v_proj(0, range(12, 16))
            av_phase(0, 2, e2)
            gather(0, 2)
            qk_proj(1, wq_sb, qt_sb)
            av_phase(0, 3, e3)
            gather(0, 3)
            qk_proj(1, wk_sb, kt_sb)
            v_proj(1, range(0, 8))
            f0 = qk_phase(1, 0)
            f1 = qk_phase(1, 1)
            av_phase(1, 0, f0)
            gather(1, 0)
            f2 = qk_phase(1, 2)
            v_proj(1, range(8, 12))
            av_phase(1, 1, f1)
            gather(1, 1)
            f3 = qk_phase(1, 3)
            v_proj(1, range(12, 16))
            av_phase(1, 2, f2)
            gather(1, 2)
            load_yg(1, 0)       # gpsimd; cc(1,0) done by dispatch time
            load_yg(0, 0)
            load_yg(0, 1)
            load_yg(0, 2)
            load_yg(0, 3)
            o_proj_tile(0, ALLCB, ogroups, True, True, defer=True)
            av_phase(1, 3, f3)
            gather(1, 3)
            # late pair-1 tiles load via the ACT DGE: its exp queue drains
            # right as these become needed, and nothing queues behind it
            load_yg(1, 1, eng=nc.scalar)
            o_proj_tile(1, ALLCB, ogroups, True, True, defer=True)
            for st, m, n in deferred_outs:
                nc.sync.dma_start(
                    out[m * 128:(m + 1) * 128, n * QT: n * QT + QT], st[:])
            deferred_outs.clear()
            load_yg(1, 2, eng=nc.scalar)
            o_proj_tile(2, ALLCB, ogroups, True, True)
            # last tile: pair-0 half first; pair-1 blocks land after the
            # final gather, split across two DGEs to halve the latency
            o_proj_tile(3, EVENCB, ogroups, True, False)
            load_yg(1, 3, split=True)
            o_proj_tile(3, ODDCB, ogroups, False, True)

    nc.compile()
    return nc


def _masks_np():
    """Diagonal causal mask: [ki, qi] = qi >= ki, duplicated along the free
    axis for the two packed heads."""
    ki = np.arange(128)[:, None]
    qi = np.arange(128)[None, :]
    tri = (qi >= ki).astype(np.float32)
    ones = np.ones((128, 128), np.float32)
    zeros = np.zeros((128, 128), np.float32)
    lo = np.concatenate([tri, ones], axis=1)    # lower k-block of a band
    hi = np.concatenate([zeros, tri], axis=1)   # upper k-block of a band
    return np.concatenate([lo, hi, lo, hi], axis=1).astype(BF16)  # [128, 1024]


def _block(a, w):
    """[C, w] -> [128, NCB*w] partition-blocked bf16."""
    return np.ascontiguousarray(
        a.reshape(NCB, 128, w).transpose(1, 0, 2).reshape(128, NCB * w)).astype(BF16)


def _prep_inputs(x, Wq, Wk, Wv, Wo, t_len):
    masks = _masks_np()
    in_maps = []
    for c in range(N_CORES):
        b, hg = divmod(c, 4)
        sl = slice(hg * SH, (hg + 1) * SH)
        in_maps.append({
            "xT": _block(x[b].T, t_len),
            "wqT": _block(Wq[sl, :].T, SH),
            "wkT": _block(Wk[sl, :].T, SH),
            "wvT": _block(Wv[sl, :].T, SH),
            "woT": _block(Wo[sl, :].T, SH),
            "masks": masks,
        })
    return in_maps


def _assemble(results, t_len):
    out = np.empty((B, t_len, C), dtype=np.float32)
    for c in range(N_CORES):
        b, hg = divmod(c, 4)
        out[b, :, hg * SH:(hg + 1) * SH] = results[c]["out"].T.astype(np.float32)
    return out


def get_nc(t_len=T):
    if t_len not in _CACHE:
        _CACHE[t_len] = _build(t_len)
    return _CACHE[t_len]


def kernel(x, Wq, Wk, Wv, Wo):
    from concourse import bass_utils
    x = np.asarray(x, dtype=np.float32)
    nc = get_nc(T)
    in_maps = _prep_inputs(x, np.asarray(Wq), np.asarray(Wk), np.asarray(Wv),
                           np.asarray(Wo), T)
    res = bass_utils.run_bass_kernel_spmd(nc, in_maps, core_ids=list(range(N_CORES)))
    return _assemble(res.results, T)


# revision 61
# speedup vs baseline: 1.0191x; 1.0057x over previous
"""Distributed causal self-attention kernel for one TRN2 chip (8 NeuronCores).

Problem: y = CausalSelfAttention(x) with B=2, T=2048, C=1024, 16 heads x 64.

Sharding (per core c = b*4 + hg;  b = batch, hg = head-group of 4 heads):
  - Q/K/V projections: column-sharded per head group (each core computes its
    4 heads' Q,K,V from the full x of its batch).
  - Attention: fully local (4 heads per core), flash-style. Scores are kept
    transposed (s^T[k, q]); the AV matmul emits y in [q, d] orientation
    (65-wide output incl. a ones-column row-sum), so the softmax denominator
    is per-partition and normalization is a cheap per-partition scalar mul.
  - y[q, d] tiles are transposed back to y^T[d, q] on the DMA engines
    (dma_start_transpose), then AllGathered within each batch group of 4
    cores (one gather per (head-pair, t-half), pipelined against compute).
  - o_proj: each core computes its own 256 output columns from the full
    gathered y^T -> output shards are disjoint; the host just concatenates.

All matmuls run in bf16 (fp32 accumulation in PSUM); inputs are converted to
bf16 on the host. QK^T matmuls (contraction dim 64) are packed two-per-PE
via tile_position row tiling.
"""
import sys
sys.path.insert(0, '/opt/trn_rl_repo')
import numpy as np
import ml_dtypes

B, T, C = 2, 2048, 1024
NH, HD = 16, 64
N_CORES = 8
GROUPS = [[0, 1, 2, 3], [4, 5, 6, 7]]
HPC = NH // 4            # heads per core = 4
SH = HPC * HD            # per-core projection width = 256
NCB = C // 128           # contraction blocks = 8
QT = 512                 # query tile
BF16 = ml_dtypes.bfloat16

_CACHE = {}


def _build(t_len):
    import concourse.bass as bass
    import concourse.bacc as bacc
    import concourse.tile as tile
    import concourse.mybir as mybir
    dt = mybir.dt
    f32, bf16 = dt.float32, dt.bfloat16

    nqt = t_len // QT        # query tiles
    ntc = t_len // 128       # t chunks of 128
    VW = HPC * 65            # vhat row width = 260

    nc = bacc.Bacc("TRN2", target_bir_lowering=False, debug=False,
                   num_devices=N_CORES)
    # inputs arrive pre-blocked on the host: [(cblk p) ...] -> [p, cblk*...]
    xT = nc.dram_tensor("xT", [128, NCB * t_len], bf16, kind="ExternalInput")
    wq = nc.dram_tensor("wqT", [128, NCB * SH], bf16, kind="ExternalInput")
    wk = nc.dram_tensor("wkT", [128, NCB * SH], bf16, kind="ExternalInput")
    wv = nc.dram_tensor("wvT", [128, NCB * SH], bf16, kind="ExternalInput")
    wo = nc.dram_tensor("woT", [128, NCB * SH], bf16, kind="ExternalInput")
    masks = nc.dram_tensor("masks", [128, 1024], bf16, kind="ExternalInput")
    out = nc.dram_tensor("out", [SH, t_len], bf16, kind="ExternalOutput")

    n_th = max(1, t_len // 1024)
    th_len = t_len // n_th
    nth = th_len // QT       # q-tiles per t-half

    with tile.TileContext(nc) as tc:
        with tc.tile_pool(name="big", bufs=1) as big, \
             tc.tile_pool(name="epool", bufs=40) as epool, \
             tc.tile_pool(name="small", bufs=8) as small, \
             tc.tile_pool(name="ygp", bufs=8) as ygp, \
             tc.tile_pool(name="stp", bufs=6) as stp, \
             tc.tile_pool(name="ps", bufs=2, space="PSUM") as psp, \
             tc.tile_pool(name="yps", bufs=2, space="PSUM") as yps, \
             tc.tile_pool(name="dram", bufs=1, space="DRAM") as dram:

            # ---- resident SBUF tensors ----
            xt = big.tile([128, NCB * t_len], bf16)       # x^T, c-blocked
            wq_sb = big.tile([128, NCB * SH], bf16)
            wk_sb = big.tile([128, NCB * SH], bf16)
            wv_sb = big.tile([128, NCB * SH], bf16)
            wo_sb = big.tile([128, NCB * SH], bf16)
            mask_sb = big.tile([128, 1024], bf16)
            qt_sb = big.tile([128, 2 * t_len], bf16)      # Q^T, pair-blocked
            kt_sb = big.tile([128, 2 * t_len], bf16)
            vhat_sb = big.tile([128, ntc * VW], bf16)     # [V_h | 1] per head
            yt_sb = big.tile([128, 2 * t_len], bf16)      # y^T, pair-blocked

            # DMA emission in first-consumer order, batched large so the
            # HWDGE's 625ns fixed issue cost stays off the critical path:
            # pair-0 Q/K projection runs k-outer, consuming each x c-block
            # for both Q and K as it lands.
            half = NCB // 2 * SH
            nc.sync.dma_start(wq_sb[:, 0:SH], wq[:, 0:SH])
            nc.sync.dma_start(wk_sb[:, 0:SH], wk[:, 0:SH])
            nc.sync.dma_start(xt[:, 0:t_len // 2], xT[:, 0:t_len // 2])
            nc.sync.dma_start(wq_sb[:, SH:half], wq[:, SH:half])
            nc.sync.dma_start(wk_sb[:, SH:half], wk[:, SH:half])
            nc.sync.dma_start(xt[:, t_len // 2:t_len], xT[:, t_len // 2:t_len])
            for k in range(1, NCB // 2):
                nc.sync.dma_start(xt[:, k * t_len:(k + 1) * t_len],
                                  xT[:, k * t_len:(k + 1) * t_len])
            nc.sync.dma_start(wq_sb[:, half:], wq[:, half:])
            nc.sync.dma_start(wk_sb[:, half:], wk[:, half:])
            for k in range(NCB // 2, NCB):
                nc.sync.dma_start(xt[:, k * t_len:(k + 1) * t_len],
                                  xT[:, k * t_len:(k + 1) * t_len])
            nc.sync.dma_start(wv_sb[:], wv[:])
            nc.sync.dma_start(mask_sb[:], masks[:])
            nc.sync.dma_start(wo_sb[:], wo[:])
            nc.gpsimd.memset(vhat_sb[:], 1.0)

            # ---- DRAM bounce buffers for the AllGathers (pair x q-tile) ----
            agin = [[dram.tile([128, QT], bf16, name=f"agin{p}{n}")
                     for n in range(nqt)] for p in (0, 1)]
            agout = [[dram.tile([512, QT], bf16, name=f"agout{p}{n}")
                      for n in range(nqt)] for p in (0, 1)]

            def qk_proj(pair, w_sb, dst_sb):
                """Q^T/K^T for one head pair: dst rows = head dims (2x64)."""
                for n in range(nqt):
                    ps = psp.tile([128, 1024], f32, name="ps")
                    for k in range(NCB):
                        nc.tensor.matmul(
                            ps[:, 0:QT],
                            lhsT=w_sb[:, k * SH + pair * 128: k * SH + (pair + 1) * 128],
                            rhs=xt[:, k * t_len + n * QT: k * t_len + n * QT + QT],
                            start=(k == 0), stop=(k == NCB - 1))
                    nc.vector.tensor_copy(
                        dst_sb[:, pair * t_len + n * QT: pair * t_len + n * QT + QT],
                        ps[:, 0:QT])

            def qk_proj0_streamed():
                """Pair-0 Q and K projections together, k-outer: every x
                c-block feeds 8 open psum groups (Q n0-3, K n0-3 -- two
                512-col groups per 2-bank tile, one group per bank) so PE
                tracks the incoming x stream instead of waiting for all of
                x before each 512-q tile."""
                tiles = [psp.tile([128, 1024], f32, name="ps") for _ in range(2)] \
                    + [yps.tile([128, 1024], f32, name="yps") for _ in range(2)]
                for k in range(NCB):
                    for i, w_sb in enumerate((wq_sb, wk_sb)):
                        for n in range(nqt):
                            ps = tiles[i * 2 + n // 2]
                            nc.tensor.matmul(
                                ps[:, (n % 2) * QT:(n % 2 + 1) * QT],
                                lhsT=w_sb[:, k * SH: k * SH + 128],
                                rhs=xt[:, k * t_len + n * QT: k * t_len + n * QT + QT],
                                start=(k == 0), stop=(k == NCB - 1),
                                skip_group_check=True)
                for n in range(nqt):
                    for i, dst_sb in enumerate((qt_sb, kt_sb)):
                        # ACT is idle until the first exp: take the psum
                        # drains there so DVE stays free for the v_proj
                        # chain. n-major order so qk_phase(0,0) can start
                        # after the first Q/K pair of copies.
                        nc.scalar.copy(
                            dst_sb[:, n * QT:(n + 1) * QT],
                            tiles[i * 2 + n // 2][:, (n % 2) * QT:(n % 2 + 1) * QT])

            def v_proj(pair, tchs, on_act=False, pool=None):
                """V for one head pair in [t, d] layout, written into vhat
                (cols h*65..h*65+63 per head; col h*65+64 stays 1). Emitted
                in chunks so it can fill PE idle while the exp stream (ACT)
                catches up with the attention matmuls."""
                pool = pool or psp
                for tch in tchs:
                    ps = pool.tile([128, 1024], f32,
                                   name="ps" if pool is psp else "yps")
                    for k in range(NCB):
                        nc.tensor.matmul(
                            ps[:, 0:128],
                            lhsT=xt[:, k * t_len + tch * 128: k * t_len + (tch + 1) * 128],
                            rhs=wv_sb[:, k * SH + pair * 128:k * SH + (pair + 1) * 128],
                            start=(k == 0), stop=(k == NCB - 1))
                    dst = vhat_sb[:, tch * VW + pair * 130: tch * VW + (pair + 1) * 130]
                    cp = nc.scalar.copy if on_act else nc.vector.tensor_copy
                    cp(dst.rearrange("p (h e) -> p h e", e=65)[:, :, 0:64],
                       ps[:, 0:128].rearrange("p (h e) -> p h e", e=64))

            def qk_phase(pair, qi):
                """QK + exp (+ causal mask) for one 512-q tile. Returns the
                e-tiles (scores stay transposed: e[k, q] = exp(s^T)) keyed so
                av_phase can slice per (k-block, 128-q block)."""
                q0 = qi * QT
                nfull = q0 // 128

                def qk_mm(dst, kb, qa, w, h01):
                    nc.tensor.matmul(
                        dst,
                        lhsT=kt_sb[h01 * 64:(h01 + 1) * 64,
                                   pair * t_len + kb * 128: pair * t_len + (kb + 1) * 128],
                        rhs=qt_sb[h01 * 64:(h01 + 1) * 64,
                                  pair * t_len + qa: pair * t_len + qa + w],
                        start=True, stop=True,
                        tile_position=(h01 * 64, 0))

                efull = []
                for kb in range(nfull):
                    qk = psp.tile([128, 1024], f32, name="ps")
                    for h01 in (0, 1):
                        qk_mm(qk[:, h01 * 512:(h01 + 1) * 512], kb, q0, 512, h01)
                    e = epool.tile([128, 1024], bf16, name="e")
                    nc.scalar.activation(e[:], qk[:],
                                         mybir.ActivationFunctionType.Exp,
                                         scale=1.0 / np.sqrt(HD))
                    efull.append(e)
                # mid supertile: blocks nfull, nfull+1 are fully valid for the
                # upper q-half [q0+256, q0+512). Packed (i, h01) x 256.
                mid = psp.tile([128, 1024], f32, name="ps")
                for i in (0, 1):
                    for h01 in (0, 1):
                        qk_mm(mid[:, (h01 * 2 + i) * 256:(h01 * 2 + i + 1) * 256],
                              nfull + i, q0 + 256, 256, h01)
                em = epool.tile([128, 1024], bf16, name="e")
                nc.scalar.activation(em[:], mid[:],
                                     mybir.ActivationFunctionType.Exp,
                                     scale=1.0 / np.sqrt(HD))
                # Two diagonal bands: band u covers q-half [q0+u*256, +256)
                # against k-blocks nfull+2u, nfull+2u+1 with the causal mask.
                ebands = []
                for u in (0, 1):
                    bd = psp.tile([128, 1024], f32, name="ps")
                    for i in (0, 1):
                        for h01 in (0, 1):
                            qk_mm(bd[:, (h01 * 2 + i) * 256:(h01 * 2 + i + 1) * 256],
                                  nfull + 2 * u + i, q0 + u * 256, 256, h01)
                    eb = epool.tile([128, 1024], bf16, name="e")
                    nc.scalar.activation(eb[:], bd[:],
                                         mybir.ActivationFunctionType.Exp,
                                         scale=1.0 / np.sqrt(HD))
                    # only the diagonal quarters (i==j) need the tri mask:
                    # the i<j quarters are fully valid, and the i>j quarters
                    # are skipped by av_phase entirely
                    ebt = eb.rearrange("p (h i j c) -> p h i j c", h=2, i=2, j=2)
                    mt = mask_sb.rearrange("p (h i j c) -> p h i j c", h=2, i=2, j=2)
                    nc.vector.tensor_mul(ebt[:, :, 0, 0], ebt[:, :, 0, 0],
                                         mt[:, :, 0, 0])
                    nc.vector.tensor_mul(ebt[:, :, 1, 1], ebt[:, :, 1, 1],
                                         mt[:, :, 1, 1])
                    ebands.append(eb)
                return efull, em, ebands

            def av_phase(pair, qi, etiles, tail=False):
                """AV + normalize + transpose for one 512-q tile.

                y accumulates in [q, d] orientation; each 128-q block qb gets
                its own [128, 1024] psum tile so the two heads' accumulation
                groups land in different banks (h01 at col 0 / 512) -- a
                start=True matmul clears has_written bits bank-wide, so one
                open group per bank is a hard rule.
                """
                efull, em, ebands = etiles
                q0 = qi * QT
                nfull = q0 // 128

                def av_mm(yq, e_ap, kb, h01, start, stop):
                    h = pair * 2 + h01
                    return nc.tensor.matmul(
                        yq[:, h01 * 512: h01 * 512 + 65],
                        lhsT=e_ap,
                        rhs=vhat_sb[:, kb * VW + h * 65: kb * VW + (h + 1) * 65],
                        start=start, stop=stop,
                        skip_group_check=True)

                def norm(yq, j):
                    """y_norm[q, 2hx64] = y / rowsum; transpose to y^T.
                    On the critical tail (ACT's exp queue already drained),
                    the h01=1 mul and odd-j transposes ride ACT so the
                    normalize chain is two-wide."""
                    rc = small.tile([128, 2], f32, name="rc")
                    nc.vector.reciprocal(
                        rc[:],
                        yq.rearrange("p (h c) -> p h c", h=2)[:, :, 64:65])
                    yn = small.tile([128, 128], bf16, name="yn")
                    for h01 in (0, 1):
                        if tail and h01 == 1:
                            nc.scalar.activation(
                                yn[:, h01 * 64:(h01 + 1) * 64],
                                yq[:, h01 * 512: h01 * 512 + 64],
                                mybir.ActivationFunctionType.Copy,
                                scale=rc[:, h01:h01 + 1])
                        else:
                            nc.vector.tensor_scalar_mul(
                                yn[:, h01 * 64:(h01 + 1) * 64],
                                yq[:, h01 * 512: h01 * 512 + 64],
                                rc[:, h01:h01 + 1])
                    tr = nc.scalar if (tail and j % 2) else nc.sync
                    tr.dma_start_transpose(
                        yt_sb[:, pair * t_len + q0 + j * 128: pair * t_len + q0 + (j + 1) * 128],
                        yn[:])

                # lower q-half: k-blocks 0..nfull-1 (full) + band0
                for j in (0, 1):
                    yq = yps.tile([128, 1024], f32, name="yps")
                    for h01 in (0, 1):
                        for kb in range(nfull):
                            av_mm(yq, efull[kb][:, h01 * 512 + j * 128: h01 * 512 + (j + 1) * 128],
                                  kb, h01, start=(kb == 0), stop=False)
                        for i in (0, 1):
                            if i == 1 and j == 0:   # fully masked quarter
                                continue
                            av_mm(yq, ebands[0][:, (h01 * 2 + i) * 256 + j * 128:
                                                 (h01 * 2 + i) * 256 + (j + 1) * 128],
                                  nfull + i, h01,
                                  start=(nfull == 0 and i == 0),
                                  stop=(i == 1 or j == 0))
                    norm(yq, j)
                # upper q-half: full + mid + band1
                for j in (0, 1):
                    yq = yps.tile([128, 1024], f32, name="yps")
                    for h01 in (0, 1):
                        for kb in range(nfull):
                            av_mm(yq, efull[kb][:, h01 * 512 + (2 + j) * 128:
                                                h01 * 512 + (3 + j) * 128],
                                  kb, h01, start=(kb == 0), stop=False)
                        for i in (0, 1):
                            av_mm(yq, em[:, (h01 * 2 + i) * 256 + j * 128:
                                          (h01 * 2 + i) * 256 + (j + 1) * 128],
                                  nfull + i, h01,
                                  start=(nfull == 0 and i == 0), stop=False)
                        for i in (0, 1):
                            if i == 1 and j == 0:   # fully masked quarter
                                continue
                            av_mm(yq, ebands[1][:, (h01 * 2 + i) * 256 + j * 128:
                                                 (h01 * 2 + i) * 256 + (j + 1) * 128],
                                  nfull + 2 + i, h01, start=False,
                                  stop=(i == 1 or j == 0))
                    norm(yq, 2 + j)

            def gather(p, n):
                """Ship one (pair, 512-q tile) of y^T to the batch group."""
                nc.sync.dma_start(
                    agin[p][n][:],
                    yt_sb[:, p * t_len + n * QT: p * t_len + (n + 1) * QT])
                nc.gpsimd.collective_compute(
                    "AllGather", mybir.AluOpType.bypass,
                    replica_groups=GROUPS,
                    ins=[agin[p][n].opt()], outs=[agout[p][n].opt()])

            ygt = {}  # (global c-block, q-tile) -> sbuf AP [128, 512]

            def load_yg(p, n, eng=None, split=False):
                """One DMA per (pair, q-tile): a 3D AP pulls all 4 ranks'
                [128, 512] blocks at once. split=True issues two halves on
                ACT + SP concurrently to halve the latency on the tail."""
                t = ygp.tile([128, 4 * QT], bf16, name="yg")
                if split:
                    for hf, e in ((0, nc.scalar), (1, nc.sync)):
                        e.dma_start(
                            t[:, hf * 2 * QT:(hf + 1) * 2 * QT].rearrange(
                                "p (r c) -> p r c", r=2),
                            agout[p][n][hf * 256:(hf + 1) * 256, :].rearrange(
                                "(r p) c -> p r c", r=2))
                else:
                    (eng or nc.gpsimd).dma_start(
                        t.rearrange("p (r c) -> p r c", r=4),
                        agout[p][n].rearrange("(r p) c -> p r c", r=4))
                for r in range(4):
                    ygt[(2 * r + p, n)] = t[:, r * QT:(r + 1) * QT]

            deferred_outs = []

            def o_proj_tile(n, cbs, groups, start, stop, defer=False):
                """One 512-t output tile (both 128-o halves), accumulating
                only c-blocks `cbs`; the psum group can stay open across
                calls (start/stop) so the pair-0 half can run before the
                last pair-1 gather lands. defer=True postpones the out DMAs
                so SP stays clear for the latency-critical gather chain."""
                for m in (0, 1):
                    if start:
                        groups[(n, m)] = psp.tile([128, 1024], f32, name="ps")
                    ps = groups[(n, m)]
                    for idx, cb in enumerate(cbs):
                        nc.tensor.matmul(
                            ps[:, 0:QT],
                            lhsT=wo_sb[:, cb * SH + m * 128: cb * SH + (m + 1) * 128],
                            rhs=ygt[(cb, n)],
                            start=(start and idx == 0),
                            stop=(stop and idx == len(cbs) - 1))
                    if stop:
                        st = stp.tile([128, 512], bf16, name="st")
                        nc.vector.tensor_copy(st[:], ps[:, 0:QT])
                        if defer:
                            deferred_outs.append((st, m, n))
                        else:
                            nc.sync.dma_start(
                                out[m * 128:(m + 1) * 128, n * QT: n * QT + QT],
                                st[:])

            # ---- schedule ----
            # Ordered so the serial, ACT-only exp stream never starves and
            # ends on the CHEAPEST tile: q-tiles go descending (the 15-block
            # tile exps while PE still has projection work; the 3-block tile
            # is last so the closing exp -> AV -> gather -> o_proj chain is
            # short). Pair-1 projections + its big qk tile are hoisted into
            # pair-0's attention so ACT crosses the pair boundary without a
            # gap. e-tiles buffer in SBUF (epool) while AV lags several
            # tiles behind QK. Collectives fire per (pair, q-tile) as soon
            # as that tile's y^T ships; o_proj tiles splice into the tail,
            # with the last tile's pair-0 half pre-accumulated under the
            # final gather.
            ogroups = {}
            ALLCB = list(range(NCB))
            EVENCB, ODDCB = [0, 2, 4, 6], [1, 3, 5, 7]

            qk_proj0_streamed()
            v_proj(0, range(0, 8), on_act=False, pool=yps)
            e0 = qk_phase(0, 0)
            e1 = qk_phase(0, 1)
            av_phase(0, 0, e0)
            gather(0, 0)
            e2 = qk_phase(0, 2)
            v_proj(0, range(8, 12))
            av_phase(0, 1, e1)
            gather(0, 1)
            e3 = qk_phase(0, 3)
            v_proj(0, range(12, 16))
            av_phase(0, 2, e2)
            gather(0, 2)
            qk_proj(1, wq_sb, qt_sb)
            av_phase(0, 3, e3)
            gather(0, 3)
            qk_proj(1, wk_sb, kt_sb)
            v_proj(1, range(0, 8))
            f0 = qk_phase(1, 0)
            f1 = qk_phase(1, 1)
            av_phase(1, 0, f0)
            gather(1, 0)
            f2 = qk_phase(1, 2)
            v_proj(1, range(8, 12))
            av_phase(1, 1, f1)
            gather(1, 1)
            f3 = qk_phase(1, 3)
            v_proj(1, range(12, 16))
            av_phase(1, 2, f2)
            gather(1, 2)
            load_yg(1, 0)       # gpsimd; cc(1,0) done by dispatch time
            load_yg(0, 0)
            load_yg(0, 1)
            load_yg(0, 2)
            load_yg(0, 3)
            o_proj_tile(0, ALLCB, ogroups, True, True, defer=True)
            av_phase(1, 3, f3)
            gather(1, 3)
            # late pair-1 tiles load via the ACT DGE: its exp queue drains
            # right as these become needed, and nothing queues behind it
            load_yg(1, 1, eng=nc.scalar)
            o_proj_tile(1, ALLCB, ogroups, True, True, defer=True)
            for st, m, n in deferred_outs:
                nc.sync.dma_start(
                    out[m * 128:(m + 1) * 128, n * QT: n * QT + QT], st[:])
            deferred_outs.clear()
            load_yg(1, 2, eng=nc.scalar)
            o_proj_tile(2, ALLCB, ogroups, True, True)
            # last tile: pair-0 half first; pair-1 blocks land after the
            # final gather, split across two DGEs to halve the latency
            o_proj_tile(3, EVENCB, ogroups, True, False)
            load_yg(1, 3, split=True)
            o_proj_tile(3, ODDCB, ogroups, False, True)

    nc.compile()
    return nc


def _masks_np():
    """Diagonal causal mask: [ki, qi] = qi >= ki, duplicated along the free
    axis for the two packed heads."""
    ki = np.arange(128)[:, None]
    qi = np.arange(128)[None, :]
    tri = (qi >= ki).astype(np.float32)
    ones = np.ones((128, 128), np.float32)
    zeros = np.zeros((128, 128), np.float32)
    lo = np.concatenate([tri, ones], axis=1)    # lower k-block of a band
    hi = np.concatenate([zeros, tri], axis=1)   # upper k-block of a band
    return np.concatenate([lo, hi, lo, hi], axis=1).astype(BF16)  # [128, 1024]


def _block(a, w):
    """[C, w] -> [128, NCB*w] partition-blocked bf16."""
    return np.ascontiguousarray(
        a.reshape(NCB, 128, w).transpose(1, 0, 2).reshape(128, NCB * w)).astype(BF16)


def _prep_inputs(x, Wq, Wk, Wv, Wo, t_len):
    masks = _masks_np()
    in_maps = []
    for c in range(N_CORES):
        b, hg = divmod(c, 4)
        sl = slice(hg * SH, (hg + 1) * SH)
        in_maps.append({
            "xT": _block(x[b].T, t_len),
            "wqT": _block(Wq[sl, :].T, SH),
            "wkT": _block(Wk[sl, :].T, SH),
            "wvT": _block(Wv[sl, :].T, SH),
            "woT": _block(Wo[sl, :].T, SH),
            "masks": masks,
        })
    return in_maps


def _assemble(results, t_len):
    out = np.empty((B, t_len, C), dtype=np.float32)
    for c in range(N_CORES):
        b, hg = divmod(c, 4)
        out[b, :, hg * SH:(hg + 1) * SH] = results[c]["out"].T.astype(np.float32)
    return out


def get_nc(t_len=T):
    if t_len not in _CACHE:
        _CACHE[t_len] = _build(t_len)
    return _CACHE[t_len]


def kernel(x, Wq, Wk, Wv, Wo):
    from concourse import bass_utils
    x = np.asarray(x, dtype=np.float32)
    nc = get_nc(T)
    in_maps = _prep_inputs(x, np.asarray(Wq), np.asarray(Wk), np.asarray(Wv),
                           np.asarray(Wo), T)
    res = bass_utils.run_bass_kernel_spmd(nc, in_maps, core_ids=list(range(N_CORES)))
    return _assemble(res.results, T)


# revision 66
# speedup vs baseline: 1.0363x; 1.0170x over previous
"""Distributed causal self-attention kernel for one TRN2 chip (8 NeuronCores).

Problem: y = CausalSelfAttention(x) with B=2, T=2048, C=1024, 16 heads x 64.

Sharding (per core c = b*4 + hg;  b = batch, hg = head-group of 4 heads):
  - Q/K/V projections: column-sharded per head group (each core computes its
    4 heads' Q,K,V from the full x of its batch).
  - Attention: fully local (4 heads per core), flash-style. Scores are kept
    transposed (s^T[k, q]); the AV matmul emits y in [q, d] orientation
    (65-wide output incl. a ones-column row-sum), so the softmax denominator
    is per-partition and normalization is a cheap per-partition scalar mul.
  - y[q, d] tiles are transposed back to y^T[d, q] on the DMA engines
    (dma_start_transpose), then AllGathered within each batch group of 4
    cores (one gather per (head-pair, t-half), pipelined against compute).
  - o_proj: each core computes its own 256 output columns from the full
    gathered y^T -> output shards are disjoint; the host just concatenates.

All matmuls run in bf16 (fp32 accumulation in PSUM); inputs are converted to
bf16 on the host. QK^T matmuls (contraction dim 64) are packed two-per-PE
via tile_position row tiling.
"""
import sys
sys.path.insert(0, '/opt/trn_rl_repo')
import numpy as np
import ml_dtypes

B, T, C = 2, 2048, 1024
NH, HD = 16, 64
N_CORES = 8
GROUPS = [[0, 1, 2, 3], [4, 5, 6, 7]]
HPC = NH // 4            # heads per core = 4
SH = HPC * HD            # per-core projection width = 256
NCB = C // 128           # contraction blocks = 8
QT = 512                 # query tile
BF16 = ml_dtypes.bfloat16

_CACHE = {}


def _build(t_len):
    import concourse.bass as bass
    import concourse.bacc as bacc
    import concourse.tile as tile
    import concourse.mybir as mybir
    dt = mybir.dt
    f32, bf16 = dt.float32, dt.bfloat16

    nqt = t_len // QT        # query tiles
    ntc = t_len // 128       # t chunks of 128
    VW = HPC * 65            # vhat row width = 260

    nc = bacc.Bacc("TRN2", target_bir_lowering=False, debug=False,
                   num_devices=N_CORES)
    # inputs arrive pre-blocked on the host: [(cblk p) ...] -> [p, cblk*...]
    xT = nc.dram_tensor("xT", [128, NCB * t_len], bf16, kind="ExternalInput")
    wq = nc.dram_tensor("wqT", [128, NCB * SH], bf16, kind="ExternalInput")
    wk = nc.dram_tensor("wkT", [128, NCB * SH], bf16, kind="ExternalInput")
    wv = nc.dram_tensor("wvT", [128, NCB * SH], bf16, kind="ExternalInput")
    wo = nc.dram_tensor("woT", [128, NCB * SH], bf16, kind="ExternalInput")
    masks = nc.dram_tensor("masks", [128, 1024], bf16, kind="ExternalInput")
    out = nc.dram_tensor("out", [SH, t_len], bf16, kind="ExternalOutput")

    n_th = max(1, t_len // 1024)
    th_len = t_len // n_th
    nth = th_len // QT       # q-tiles per t-half

    with tile.TileContext(nc) as tc:
        with tc.tile_pool(name="big", bufs=1) as big, \
             tc.tile_pool(name="epool", bufs=40) as epool, \
             tc.tile_pool(name="small", bufs=8) as small, \
             tc.tile_pool(name="ygp", bufs=8) as ygp, \
             tc.tile_pool(name="stp", bufs=6) as stp, \
             tc.tile_pool(name="ps", bufs=2, space="PSUM") as psp, \
             tc.tile_pool(name="yps", bufs=2, space="PSUM") as yps, \
             tc.tile_pool(name="dram", bufs=1, space="DRAM") as dram:

            # ---- resident SBUF tensors ----
            xt = big.tile([128, NCB * t_len], bf16)       # x^T, c-blocked
            wq_sb = big.tile([128, NCB * SH], bf16)
            wk_sb = big.tile([128, NCB * SH], bf16)
            wv_sb = big.tile([128, NCB * SH], bf16)
            wo_sb = big.tile([128, NCB * SH], bf16)
            mask_sb = big.tile([128, 1024], bf16)
            qt_sb = big.tile([128, 2 * t_len], bf16)      # Q^T, pair-blocked
            kt_sb = big.tile([128, 2 * t_len], bf16)
            vhat_sb = big.tile([128, ntc * VW], bf16)     # [V_h | 1] per head
            yt_sb = big.tile([128, 2 * t_len], bf16)      # y^T, pair-blocked

            # DMA emission in first-consumer order, batched large so the
            # HWDGE's 625ns fixed issue cost stays off the critical path:
            # pair-0 Q/K projection runs k-outer, consuming each x c-block
            # for both Q and K as it lands.
            half = NCB // 2 * SH
            nc.sync.dma_start(wq_sb[:, 0:SH], wq[:, 0:SH])
            nc.sync.dma_start(wk_sb[:, 0:SH], wk[:, 0:SH])
            nc.sync.dma_start(xt[:, 0:t_len // 2], xT[:, 0:t_len // 2])
            nc.sync.dma_start(wq_sb[:, SH:half], wq[:, SH:half])
            nc.sync.dma_start(wk_sb[:, SH:half], wk[:, SH:half])
            nc.sync.dma_start(xt[:, t_len // 2:t_len], xT[:, t_len // 2:t_len])
            for k in range(1, NCB // 2):
                nc.sync.dma_start(xt[:, k * t_len:(k + 1) * t_len],
                                  xT[:, k * t_len:(k + 1) * t_len])
            nc.sync.dma_start(wq_sb[:, half:], wq[:, half:])
            nc.sync.dma_start(wk_sb[:, half:], wk[:, half:])
            for k in range(NCB // 2, NCB):
                nc.sync.dma_start(xt[:, k * t_len:(k + 1) * t_len],
                                  xT[:, k * t_len:(k + 1) * t_len])
            nc.sync.dma_start(wv_sb[:], wv[:])
            nc.sync.dma_start(mask_sb[:], masks[:])
            nc.sync.dma_start(wo_sb[:], wo[:])
            nc.gpsimd.memset(vhat_sb[:], 1.0)

            # ---- DRAM bounce buffers for the AllGathers (pair x q-tile) ----
            agin = [[dram.tile([128, QT], bf16, name=f"agin{p}{n}")
                     for n in range(nqt)] for p in (0, 1)]
            agout = [[dram.tile([512, QT], bf16, name=f"agout{p}{n}")
                      for n in range(nqt)] for p in (0, 1)]

            def qk_proj(pair, w_sb, dst_sb):
                """Q^T/K^T for one head pair: dst rows = head dims (2x64)."""
                for n in range(nqt):
                    ps = psp.tile([128, 1024], f32, name="ps")
                    for k in range(NCB):
                        nc.tensor.matmul(
                            ps[:, 0:QT],
                            lhsT=w_sb[:, k * SH + pair * 128: k * SH + (pair + 1) * 128],
                            rhs=xt[:, k * t_len + n * QT: k * t_len + n * QT + QT],
                            start=(k == 0), stop=(k == NCB - 1))
                    nc.vector.tensor_copy(
                        dst_sb[:, pair * t_len + n * QT: pair * t_len + n * QT + QT],
                        ps[:, 0:QT])

            def qk_proj0_streamed():
                """Pair-0 Q and K projections together, k-outer: every x
                c-block feeds 8 open psum groups (Q n0-3, K n0-3 -- two
                512-col groups per 2-bank tile, one group per bank) so PE
                tracks the incoming x stream instead of waiting for all of
                x before each 512-q tile."""
                tiles = [psp.tile([128, 1024], f32, name="ps") for _ in range(2)] \
                    + [yps.tile([128, 1024], f32, name="yps") for _ in range(2)]
                for k in range(NCB):
                    for i, w_sb in enumerate((wq_sb, wk_sb)):
                        for n in range(nqt):
                            ps = tiles[i * 2 + n // 2]
                            nc.tensor.matmul(
                                ps[:, (n % 2) * QT:(n % 2 + 1) * QT],
                                lhsT=w_sb[:, k * SH: k * SH + 128],
                                rhs=xt[:, k * t_len + n * QT: k * t_len + n * QT + QT],
                                start=(k == 0), stop=(k == NCB - 1),
                                skip_group_check=True)
                for n in range(nqt):
                    for i, dst_sb in enumerate((qt_sb, kt_sb)):
                        # ACT is idle until the first exp: take the psum
                        # drains there so DVE stays free for the v_proj
                        # chain. n-major order so qk_phase(0,0) can start
                        # after the first Q/K pair of copies.
                        nc.scalar.copy(
                            dst_sb[:, n * QT:(n + 1) * QT],
                            tiles[i * 2 + n // 2][:, (n % 2) * QT:(n % 2 + 1) * QT])

            def v_proj(pair, tchs, on_act=False, pool=None):
                """V for one head pair in [t, d] layout, written into vhat
                (cols h*65..h*65+63 per head; col h*65+64 stays 1). Emitted
                in chunks so it can fill PE idle while the exp stream (ACT)
                catches up with the attention matmuls."""
                pool = pool or psp
                for tch in tchs:
                    ps = pool.tile([128, 1024], f32,
                                   name="ps" if pool is psp else "yps")
                    for k in range(NCB):
                        nc.tensor.matmul(
                            ps[:, 0:128],
                            lhsT=xt[:, k * t_len + tch * 128: k * t_len + (tch + 1) * 128],
                            rhs=wv_sb[:, k * SH + pair * 128:k * SH + (pair + 1) * 128],
                            start=(k == 0), stop=(k == NCB - 1))
                    dst = vhat_sb[:, tch * VW + pair * 130: tch * VW + (pair + 1) * 130]
                    cp = nc.scalar.copy if on_act else nc.vector.tensor_copy
                    cp(dst.rearrange("p (h e) -> p h e", e=65)[:, :, 0:64],
                       ps[:, 0:128].rearrange("p (h e) -> p h e", e=64))

            def qk_phase(pair, qi):
                """QK + exp (+ causal mask) for one 512-q tile. Returns the
                e-tiles (scores stay transposed: e[k, q] = exp(s^T)) keyed so
                av_phase can slice per (k-block, 128-q block)."""
                q0 = qi * QT
                nfull = q0 // 128

                def qk_mm(dst, kb, qa, w, h01):
                    nc.tensor.matmul(
                        dst,
                        lhsT=kt_sb[h01 * 64:(h01 + 1) * 64,
                                   pair * t_len + kb * 128: pair * t_len + (kb + 1) * 128],
                        rhs=qt_sb[h01 * 64:(h01 + 1) * 64,
                                  pair * t_len + qa: pair * t_len + qa + w],
                        start=True, stop=True,
                        tile_position=(h01 * 64, 0))

                efull = []
                for kb in range(nfull):
                    qk = psp.tile([128, 1024], f32, name="ps")
                    for h01 in (0, 1):
                        qk_mm(qk[:, h01 * 512:(h01 + 1) * 512], kb, q0, 512, h01)
                    e = epool.tile([128, 1024], bf16, name="e")
                    nc.scalar.activation(e[:], qk[:],
                                         mybir.ActivationFunctionType.Exp,
                                         scale=1.0 / np.sqrt(HD))
                    efull.append(e)
                # mid supertile: blocks nfull, nfull+1 are fully valid for the
                # upper q-half [q0+256, q0+512). Packed (i, h01) x 256.
                mid = psp.tile([128, 1024], f32, name="ps")
                for i in (0, 1):
                    for h01 in (0, 1):
                        qk_mm(mid[:, (h01 * 2 + i) * 256:(h01 * 2 + i + 1) * 256],
                              nfull + i, q0 + 256, 256, h01)
                em = epool.tile([128, 1024], bf16, name="e")
                nc.scalar.activation(em[:], mid[:],
                                     mybir.ActivationFunctionType.Exp,
                                     scale=1.0 / np.sqrt(HD))
                # Two diagonal bands: band u covers q-half [q0+u*256, +256)
                # against k-blocks nfull+2u, nfull+2u+1 with the causal mask.
                ebands = []
                for u in (0, 1):
                    bd = psp.tile([128, 1024], f32, name="ps")
                    for i in (0, 1):
                        for h01 in (0, 1):
                            qk_mm(bd[:, (h01 * 2 + i) * 256:(h01 * 2 + i + 1) * 256],
                                  nfull + 2 * u + i, q0 + u * 256, 256, h01)
                    eb = epool.tile([128, 1024], bf16, name="e")
                    nc.scalar.activation(eb[:], bd[:],
                                         mybir.ActivationFunctionType.Exp,
                                         scale=1.0 / np.sqrt(HD))
                    # only the diagonal quarters (i==j) need the tri mask:
                    # the i<j quarters are fully valid, and the i>j quarters
                    # are skipped by av_phase entirely
                    ebt = eb.rearrange("p (h i j c) -> p h i j c", h=2, i=2, j=2)
                    mt = mask_sb.rearrange("p (h i j c) -> p h i j c", h=2, i=2, j=2)
                    nc.vector.tensor_mul(ebt[:, :, 0, 0], ebt[:, :, 0, 0],
                                         mt[:, :, 0, 0])
                    nc.vector.tensor_mul(ebt[:, :, 1, 1], ebt[:, :, 1, 1],
                                         mt[:, :, 1, 1])
                    ebands.append(eb)
                return efull, em, ebands

            def av_phase(pair, qi, etiles, tail=False):
                """AV + normalize + transpose for one 512-q tile.

                y accumulates in [q, d] orientation; each 128-q block qb gets
                its own [128, 1024] psum tile so the two heads' accumulation
                groups land in different banks (h01 at col 0 / 512) -- a
                start=True matmul clears has_written bits bank-wide, so one
                open group per bank is a hard rule.
                """
                efull, em, ebands = etiles
                q0 = qi * QT
                nfull = q0 // 128

                def av_mm(yq, e_ap, kb, h01, start, stop):
                    h = pair * 2 + h01
                    return nc.tensor.matmul(
                        yq[:, h01 * 512: h01 * 512 + 65],
                        lhsT=e_ap,
                        rhs=vhat_sb[:, kb * VW + h * 65: kb * VW + (h + 1) * 65],
                        start=start, stop=stop,
                        skip_group_check=True)

                def norm(yq, j):
                    """y_norm[q, 2hx64] = y / rowsum; transpose to y^T.
                    On the critical tail (ACT's exp queue already drained),
                    the h01=1 mul and odd-j transposes ride ACT so the
                    normalize chain is two-wide."""
                    rc = small.tile([128, 2], f32, name="rc")
                    nc.vector.reciprocal(
                        rc[:],
                        yq.rearrange("p (h c) -> p h c", h=2)[:, :, 64:65])
                    yn = small.tile([128, 128], bf16, name="yn")
                    for h01 in (0, 1):
                        if tail and h01 == 1:
                            nc.scalar.activation(
                                yn[:, h01 * 64:(h01 + 1) * 64],
                                yq[:, h01 * 512: h01 * 512 + 64],
                                mybir.ActivationFunctionType.Copy,
                                scale=rc[:, h01:h01 + 1])
                        else:
                            nc.vector.tensor_scalar_mul(
                                yn[:, h01 * 64:(h01 + 1) * 64],
                                yq[:, h01 * 512: h01 * 512 + 64],
                                rc[:, h01:h01 + 1])
                    tr = nc.scalar if (tail and j % 2) else nc.sync
                    tr.dma_start_transpose(
                        yt_sb[:, pair * t_len + q0 + j * 128: pair * t_len + q0 + (j + 1) * 128],
                        yn[:])

                # lower q-half: k-blocks 0..nfull-1 (full) + band0
                for j in (0, 1):
                    yq = yps.tile([128, 1024], f32, name="yps")
                    for h01 in (0, 1):
                        for kb in range(nfull):
                            av_mm(yq, efull[kb][:, h01 * 512 + j * 128: h01 * 512 + (j + 1) * 128],
                                  kb, h01, start=(kb == 0), stop=False)
                        for i in (0, 1):
                            if i == 1 and j == 0:   # fully masked quarter
                                continue
                            av_mm(yq, ebands[0][:, (h01 * 2 + i) * 256 + j * 128:
                                                 (h01 * 2 + i) * 256 + (j + 1) * 128],
                                  nfull + i, h01,
                                  start=(nfull == 0 and i == 0),
                                  stop=(i == 1 or j == 0))
                    norm(yq, j)
                # upper q-half: full + mid + band1
                for j in (0, 1):
                    yq = yps.tile([128, 1024], f32, name="yps")
                    for h01 in (0, 1):
                        for kb in range(nfull):
                            av_mm(yq, efull[kb][:, h01 * 512 + (2 + j) * 128:
                                                h01 * 512 + (3 + j) * 128],
                                  kb, h01, start=(kb == 0), stop=False)
                        for i in (0, 1):
                            av_mm(yq, em[:, (h01 * 2 + i) * 256 + j * 128:
                                          (h01 * 2 + i) * 256 + (j + 1) * 128],
                                  nfull + i, h01,
                                  start=(nfull == 0 and i == 0), stop=False)
                        for i in (0, 1):
                            if i == 1 and j == 0:   # fully masked quarter
                                continue
                            av_mm(yq, ebands[1][:, (h01 * 2 + i) * 256 + j * 128:
                                                 (h01 * 2 + i) * 256 + (j + 1) * 128],
                                  nfull + 2 + i, h01, start=False,
                                  stop=(i == 1 or j == 0))
                    norm(yq, 2 + j)

            def gather(p, n):
                """Ship one (pair, 512-q tile) of y^T to the batch group."""
                nc.sync.dma_start(
                    agin[p][n][:],
                    yt_sb[:, p * t_len + n * QT: p * t_len + (n + 1) * QT])
                nc.gpsimd.collective_compute(
                    "AllGather", mybir.AluOpType.bypass,
                    replica_groups=GROUPS,
                    ins=[agin[p][n].opt()], outs=[agout[p][n].opt()])

            ygt = {}  # (global c-block, q-tile) -> sbuf AP [128, 512]

            def load_yg(p, n, eng=None, split=False):
                """One DMA per (pair, q-tile): a 3D AP pulls all 4 ranks'
                [128, 512] blocks at once. split=True issues two halves on
                ACT + SP concurrently to halve the latency on the tail."""
                t = ygp.tile([128, 4 * QT], bf16, name="yg")
                if split:
                    for hf, e in ((0, nc.scalar), (1, nc.sync)):
                        e.dma_start(
                            t[:, hf * 2 * QT:(hf + 1) * 2 * QT].rearrange(
                                "p (r c) -> p r c", r=2),
                            agout[p][n][hf * 256:(hf + 1) * 256, :].rearrange(
                                "(r p) c -> p r c", r=2))
                else:
                    (eng or nc.gpsimd).dma_start(
                        t.rearrange("p (r c) -> p r c", r=4),
                        agout[p][n].rearrange("(r p) c -> p r c", r=4))
                for r in range(4):
                    ygt[(2 * r + p, n)] = t[:, r * QT:(r + 1) * QT]

            deferred_outs = []

            def o_proj_tile(n, cbs, groups, start, stop, defer=False):
                """One 512-t output tile (both 128-o halves), accumulating
                only c-blocks `cbs`; the psum group can stay open across
                calls (start/stop) so the pair-0 half can run before the
                last pair-1 gather lands. defer=True postpones the out DMAs
                so SP stays clear for the latency-critical gather chain."""
                for m in (0, 1):
                    if start:
                        groups[(n, m)] = psp.tile([128, 1024], f32, name="ps")
                    ps = groups[(n, m)]
                    for idx, cb in enumerate(cbs):
                        nc.tensor.matmul(
                            ps[:, 0:QT],
                            lhsT=wo_sb[:, cb * SH + m * 128: cb * SH + (m + 1) * 128],
                            rhs=ygt[(cb, n)],
                            start=(start and idx == 0),
                            stop=(stop and idx == len(cbs) - 1))
                    if stop:
                        st = stp.tile([128, 512], bf16, name="st")
                        nc.vector.tensor_copy(st[:], ps[:, 0:QT])
                        if defer:
                            deferred_outs.append((st, m, n))
                        else:
                            nc.sync.dma_start(
                                out[m * 128:(m + 1) * 128, n * QT: n * QT + QT],
                                st[:])

            # ---- schedule ----
            # Ordered so the serial, ACT-only exp stream never starves and
            # ends on the CHEAPEST tile: q-tiles go descending (the 15-block
            # tile exps while PE still has projection work; the 3-block tile
            # is last so the closing exp -> AV -> gather -> o_proj chain is
            # short). Pair-1 projections + its big qk tile are hoisted into
            # pair-0's attention so ACT crosses the pair boundary without a
            # gap. e-tiles buffer in SBUF (epool) while AV lags several
            # tiles behind QK. Collectives fire per (pair, q-tile) as soon
            # as that tile's y^T ships; o_proj tiles splice into the tail,
            # with the last tile's pair-0 half pre-accumulated under the
            # final gather.
            ogroups = {}
            ALLCB = list(range(NCB))
            EVENCB, ODDCB = [0, 2, 4, 6], [1, 3, 5, 7]

            qk_proj0_streamed()
            v_proj(0, range(0, 8), on_act=False, pool=yps)
            e0 = qk_phase(0, 0)
            e1 = qk_phase(0, 1)
            av_phase(0, 0, e0)
            gather(0, 0)
            e2 = qk_phase(0, 2)
            v_proj(0, range(8, 12))
            av_phase(0, 1, e1)
            gather(0, 1)
            e3 = qk_phase(0, 3)
            v_proj(0, range(12, 16))
            av_phase(0, 2, e2)
            gather(0, 2)
            qk_proj(1, wq_sb, qt_sb)
            av_phase(0, 3, e3)
            gather(0, 3)
            qk_proj(1, wk_sb, kt_sb)
            v_proj(1, range(0, 8))
            f0 = qk_phase(1, 0)
            f1 = qk_phase(1, 1)
            av_phase(1, 0, f0)
            gather(1, 0)
            f2 = qk_phase(1, 2)
            v_proj(1, range(8, 12))
            av_phase(1, 1, f1)
            gather(1, 1)
            f3 = qk_phase(1, 3)
            v_proj(1, range(12, 16))
            av_phase(1, 2, f2)
            gather(1, 2)
            load_yg(1, 0)       # gpsimd; cc(1,0) done by dispatch time
            load_yg(0, 0)
            load_yg(0, 1)
            load_yg(0, 2)
            load_yg(0, 3)
            o_proj_tile(0, ALLCB, ogroups, True, True, defer=True)
            av_phase(1, 3, f3)
            gather(1, 3)
            # late pair-1 tiles load via the ACT DGE: its exp queue drains
            # right as these become needed, and nothing queues behind it
            load_yg(1, 1, eng=nc.scalar)
            o_proj_tile(1, ALLCB, ogroups, True, True, defer=True)
            for st, m, n in deferred_outs:
                nc.sync.dma_start(
                    out[m * 128:(m + 1) * 128, n * QT: n * QT + QT], st[:])
            deferred_outs.clear()
            load_yg(1, 2, eng=nc.scalar)
            o_proj_tile(2, ALLCB, ogroups, True, True)
            # last tile: pair-0 half first; pair-1 blocks land after the
            # final gather, split across two DGEs to halve the latency
            o_proj_tile(3, EVENCB, ogroups, True, False)
            load_yg(1, 3, split=True)
            # keep PE's p-state warm across the final gather wait: a chain
            # of throwaway matmuls, each gated on a DVE drain of the previous
            # one (~600ns/link), so PE never idles long enough to reset its
            # clock ramp before the last o_proj half (results never consumed;
            # psp is held by the open o3-evens groups -> ride the yps slots)
            ws = small.tile([128, 128], bf16, name="ws")
            nc.vector.tensor_copy(ws[:], xt[:, 0:128])
            for _ in range(12):
                wp = yps.tile([128, 1024], f32, name="yps")
                nc.tensor.matmul(wp[:, 0:128], lhsT=wq_sb[:, 0:128],
                                 rhs=ws[:], start=True, stop=True)
                ws = small.tile([128, 128], bf16, name="ws")
                nc.vector.tensor_copy(ws[:], wp[:, 0:128])
            o_proj_tile(3, ODDCB, ogroups, False, True)

    nc.compile()
    return nc


def _masks_np():
    """Diagonal causal mask: [ki, qi] = qi >= ki, duplicated along the free
    axis for the two packed heads."""
    ki = np.arange(128)[:, None]
    qi = np.arange(128)[None, :]
    tri = (qi >= ki).astype(np.float32)
    ones = np.ones((128, 128), np.float32)
    zeros = np.zeros((128, 128), np.float32)
    lo = np.concatenate([tri, ones], axis=1)    # lower k-block of a band
    hi = np.concatenate([zeros, tri], axis=1)   # upper k-block of a band
    return np.concatenate([lo, hi, lo, hi], axis=1).astype(BF16)  # [128, 1024]


def _block(a, w):
    """[C, w] -> [128, NCB*w] partition-blocked bf16."""
    return np.ascontiguousarray(
        a.reshape(NCB, 128, w).transpose(1, 0, 2).reshape(128, NCB * w)).astype(BF16)


def _prep_inputs(x, Wq, Wk, Wv, Wo, t_len):
    masks = _masks_np()
    in_maps = []
    for c in range(N_CORES):
        b, hg = divmod(c, 4)
        sl = slice(hg * SH, (hg + 1) * SH)
        in_maps.append({
            "xT": _block(x[b].T, t_len),
            "wqT": _block(Wq[sl, :].T, SH),
            "wkT": _block(Wk[sl, :].T, SH),
            "wvT": _block(Wv[sl, :].T, SH),
            "woT": _block(Wo[sl, :].T, SH),
            "masks": masks,
        })
    return in_maps


def _assemble(results, t_len):
    out = np.empty((B, t_len, C), dtype=np.float32)
    for c in range(N_CORES):
        b, hg = divmod(c, 4)
        out[b, :, hg * SH:(hg + 1) * SH] = results[c]["out"].T.astype(np.float32)
    return out


def get_nc(t_len=T):
    if t_len not in _CACHE:
        _CACHE[t_len] = _build(t_len)
    return _CACHE[t_len]


def kernel(x, Wq, Wk, Wv, Wo):
    from concourse import bass_utils
    x = np.asarray(x, dtype=np.float32)
    nc = get_nc(T)
    in_maps = _prep_inputs(x, np.asarray(Wq), np.asarray(Wk), np.asarray(Wv),
                           np.asarray(Wo), T)
    res = bass_utils.run_bass_kernel_spmd(nc, in_maps, core_ids=list(range(N_CORES)))
    return _assemble(res.results, T)
